# revision 86
# baseline (speedup 1.0000x reference)
"""Fused multi-head attention layer (RoPE + ALiBi + softmax + out-proj) on 8 TRN2 cores.

Sharding: core c -> (batch b = c//2, query-half s = c%2). Each core owns 1024
queries of its batch (two 512-blocks, interleaved for ALiBi load balance),
computes K/V for all 2048 positions, and writes a disjoint slice of the output.
No collectives. All 8 cores run one SPMD graph; per-core differences (which
query blocks, ALiBi band offsets) are encoded purely in host-prepared data.

v2 optimizations (vs baseline, TimelineSim 222.6us -> 156.2us):
- Score/bias/exp/AV column-trimmed to the ALiBi-needed query prefix per
  (head-pair, key-tile, slot); bias matmuls further restricted to the
  biased suffix via split accumulation groups.
- Both heads of a pair share one [128,1024] score PSUM -> one exp
  activation per (pair, jt, slot), halving ACT fixed overhead.
- AV uses stationary=exp-tile / moving=v (65 cols) instead of streaming
  512 query columns: ~2x less PE stream time; output lands [query, dim].
- Softmax normalization becomes a per-partition tensor_scalar multiply.
- Attention tiles transposed on the DMA engines (xbar), projection runs
  with 128-deep contraction (head pairs packed), proj bias folded in as a
  rank-1 ones-row matmul.
- RoPE rotate_half applied as a +-1 permutation matmul on the projected
  q/k instead of a second rotated-weight projection (drops 96 projection
  matmuls); the psum->sbuf hop also makes the cos-multiply all-bf16.
- Deep software pipelining: score streams run one (t, slot) at a time
  with a five-deep exp/av stagger, chained across stream boundaries; QK
  chunks of the next head pair and the first half of the projection are
  injected as fillers into score-phase PE slack (on the ps_a pool, so
  they never contend with score psums).
- Inputs land in a few large multi-dim DMAs ordered by first consumer;
  the tail transposes bypass the DMA queue via PE transpose; V-phase
  psums alternate between both psum pools so the ring is never
  copy-latency bound; the final output DMAs are paired to halve the
  serialized HWDGE windows on the tail; a throwaway warmup matmul chain
  starts the PE p-state ramp clock early so real matmuls never pay the
  cold-clock penalty; prephase psum->sbuf copies split across DVE/ACT
  and the prephase rotate matmuls deferred one chunk.

(fp8 DoubleRow for the bias matmul halves its PE cost in the cost model
and passes CoreSim numerically, but the axon PJRT execution path fails
on it at runtime, so it stays bf16.)
"""

import functools
import os
import sys

import numpy as np

sys.path.insert(0, "/opt/trn_rl_repo")

import ml_dtypes  # noqa: E402

import concourse.bass as bass  # noqa: E402
import concourse.tile as tile  # noqa: E402
from concourse import bacc, mybir, bass_utils  # noqa: E402

BF16 = mybir.dt.bfloat16
F32 = mybir.dt.float32
NPBF = ml_dtypes.bfloat16

B, N, C, H, D = 4, 2048, 512, 8, 64
NCORES = 8
NQ = 1024            # local queries per core
JT = N // 128        # 16 j-tiles of 128 key positions
T_CUT = 30.0         # ALiBi cutoff in logits: exp(-30) is negligible
SCALE = D ** -0.5

# c8_h = alibi_slope_h * MAX_BIAS = 2^-(h+1) * 8 = 2^(2-h)
C8 = [2.0 ** (2 - h) for h in range(H)]
# band reach (in key positions) per head
RADIUS = [T_CUT / c for c in C8]

# SPMD union bounds over the two cores sharing a slot index:
# slot0 owns blocks {0,1}*512, slot1 owns {2,3}*512.
I0MIN = [0, 1024]
I0MAX = [512, 1536]


def _qhi(t, slot, jt):
    """Needed query-column prefix of the [128 keys x 512 q] tile (pair union)."""
    return max(
        max(0, min(512, 128 * jt + 127 + int(RADIUS[h]) + 1 - I0MIN[slot]))
        for h in (2 * t, 2 * t + 1))


def _qlo(slot, jt):
    """First query column where ALiBi bias can be nonzero (union over cores)."""
    return max(0, min(512, 128 * jt + 1 - I0MAX[slot]))


QHI = [[[_qhi(t, s, jt) for jt in range(JT)] for s in range(2)] for t in range(4)]
QLO = [[_qlo(s, jt) for jt in range(JT)] for s in range(2)]
# per (t, slot, qg): last (smallest) jt in descending order that writes qg
JSTOP = [[[min(jt for jt in range(JT) if QHI[t][s][jt] > 128 * qg)
           for qg in range(4)] for s in range(2)] for t in range(4)]

LAST_RESULT = None  # test harness reads exec_time_ns from here


def _owned_blocks(s):
    # 512-query blocks of the batch owned by query-half s (balanced for ALiBi)
    return (0, 3) if s == 0 else (1, 2)


def _rope_tables():
    inv = 1.0 / (10000.0 ** (np.arange(0, D, 2, dtype=np.float32) / D))
    f = np.arange(N, dtype=np.float32)[:, None] * inv[None, :]
    sin = np.concatenate([np.sin(f), np.sin(f)], axis=-1).astype(np.float32)
    cos = np.concatenate([np.cos(f), np.cos(f)], axis=-1).astype(np.float32)
    return sin, cos  # [N, D]


def _shared_inputs(qkv_w, proj_w, proj_b):
    wqT = np.ascontiguousarray(qkv_w[0:C].T) * SCALE       # [C, C]
    wkT = np.ascontiguousarray(qkv_w[C:2 * C].T)
    wvT = np.ascontiguousarray(qkv_w[2 * C:3 * C].T)
    wcat = np.concatenate([wqT, wkT, wvT], axis=1).astype(NPBF)

    ident128 = np.eye(128, dtype=np.float32)

    # rotate_half as a +-1 permutation: out[i] = -in[32+i], out[32+i] = in[i]
    # per 64-dim head; lhsT layout [d_in, d_out].
    rotperm = np.zeros((128, 128), np.float32)
    for hh in range(2):
        for i in range(32):
            rotperm[hh * 64 + 32 + i, hh * 64 + i] = -1.0
            rotperm[hh * 64 + i, hh * 64 + 32 + i] = 1.0

    c8eye = np.zeros((H, 128, 128), np.float32)
    for h in range(H):
        np.fill_diagonal(c8eye[h], C8[h])

    sin, cos = _rope_tables()
    cos2k = np.tile(cos.T, (2, 1))                         # [128, N]
    sin2k = np.tile(sin.T, (2, 1))
    return {
        "wcat": wcat,
        "rotperm": rotperm.astype(NPBF),
        "ident128": ident128.astype(NPBF),
        "c8eye": c8eye.astype(NPBF),
        "projwt": np.ascontiguousarray(proj_w.T).astype(NPBF),
        "biasrow": proj_b[None, :].astype(NPBF),
        "cos2k": cos2k.astype(NPBF), "sin2k": sin2k.astype(NPBF),
    }, sin, cos


def _pats_for(i0):
    jl = np.arange(128, dtype=np.float32)[:, None]
    il = np.arange(512, dtype=np.float32)[None, :]
    return [np.minimum((jt * 128 + jl) - (i0 + il), 0.0).astype(NPBF)
            for jt in range(16)]


def _core_inputs(c, x, shared, sin, cos):
    b, s = c // 2, c % 2
    blocks = _owned_blocks(s)
    gi = np.concatenate([np.arange(blk * 512, (blk + 1) * 512) for blk in blocks])

    xt = np.ascontiguousarray(x[b].T)                      # [C, N]
    xtq = np.ascontiguousarray(x[b][gi].T)                 # [C, NQ]

    cos2q = np.tile(cos[gi].T, (2, 1))                     # [128, NQ]
    sin2q = np.tile(sin[gi].T, (2, 1))

    pats0 = np.stack(_pats_for(blocks[0] * 512)[:8])
    pats1 = np.stack(_pats_for(blocks[1] * 512))

    return {
        "xt": xt.astype(NPBF),
        "xtq": xtq.astype(NPBF),
        "cos2q": cos2q.astype(NPBF), "sin2q": sin2q.astype(NPBF),
        "pats0": pats0,
        "pats1": pats1,
        **shared,
    }


def _build_graph():
    nc = bacc.Bacc("TRN2", target_bir_lowering=False, debug=False,
                   num_devices=NCORES)

    xt_d = nc.dram_tensor("xt", [C, N], BF16, kind="ExternalInput").ap()
    xtq_d = nc.dram_tensor("xtq", [C, NQ], BF16, kind="ExternalInput").ap()
    wcat_d = nc.dram_tensor("wcat", [C, 3 * C], BF16, kind="ExternalInput").ap()
    rotperm_d = nc.dram_tensor("rotperm", [128, 128], BF16, kind="ExternalInput").ap()
    ident_d = nc.dram_tensor("ident128", [128, 128], BF16, kind="ExternalInput").ap()
    cos2q_d = nc.dram_tensor("cos2q", [128, NQ], BF16, kind="ExternalInput").ap()
    sin2q_d = nc.dram_tensor("sin2q", [128, NQ], BF16, kind="ExternalInput").ap()
    cos2k_d = nc.dram_tensor("cos2k", [128, N], BF16, kind="ExternalInput").ap()
    sin2k_d = nc.dram_tensor("sin2k", [128, N], BF16, kind="ExternalInput").ap()
    pats0_d = nc.dram_tensor("pats0", [8, 128, 512], BF16, kind="ExternalInput").ap()
    pats1_d = nc.dram_tensor("pats1", [16, 128, 512], BF16, kind="ExternalInput").ap()
    c8eye_d = nc.dram_tensor("c8eye", [H, 128, 128], BF16, kind="ExternalInput").ap()
    projwt_d = nc.dram_tensor("projwt", [C, C], BF16, kind="ExternalInput").ap()
    biasrow_d = nc.dram_tensor("biasrow", [1, 512], BF16, kind="ExternalInput").ap()
    out_d = nc.dram_tensor("out", [NQ, C], F32, kind="ExternalOutput").ap()

    with tile.TileContext(nc) as tc:
        _body(nc, tc, xt_d, xtq_d, wcat_d, rotperm_d, ident_d, cos2q_d,
              sin2q_d, cos2k_d, sin2k_d, pats0_d, pats1_d, c8eye_d, projwt_d,
              biasrow_d, out_d)
    nc.compile()
    return nc


def _body(nc, tc, xt_d, xtq_d, wcat_d, rotperm_d, ident_d, cos2q_d,
          sin2q_d, cos2k_d, sin2k_d, pats0_d, pats1_d, c8eye_d, projwt_d,
          biasrow_d, out_d):
    from contextlib import ExitStack
    ctx = ExitStack()
    persist = ctx.enter_context(tc.tile_pool(name="persist", bufs=1))
    tmp_pool = ctx.enter_context(tc.tile_pool(name="ropetmp", bufs=6))
    exp_pool = ctx.enter_context(tc.tile_pool(name="exp", bufs=10))
    fin_pool = ctx.enter_context(tc.tile_pool(name="final", bufs=2))
    att_pool = ctx.enter_context(tc.tile_pool(name="att", bufs=8))
    rec_pool = ctx.enter_context(tc.tile_pool(name="rec", bufs=4))
    # PSUM: ps_s = 2 bufs x [128,1024] f32 (2 banks each); ps_a = 4 bufs x
    # [128,512] f32 (1 bank each) shared by QKV-phase psums and AV accums.
    ps_s = ctx.enter_context(tc.tile_pool(name="ps_s", bufs=2, space="PSUM"))
    ps_a = ctx.enter_context(tc.tile_pool(name="ps_a", bufs=4, space="PSUM"))

    def ptile(shape, dtype, tag):
        return persist.tile(shape, dtype, tag=tag, name=tag)

    Exp = mybir.ActivationFunctionType.Exp

    # PE p-state warmup: the cost model ramps 0.65 -> 1.2 -> 2.4 GHz over
    # ~3us of continuous execution. A chain of throwaway matmuls (dependent
    # only on an early memset) starts the ramp clock at ~0.3us so the first
    # real V matmuls already run at full clock.
    ones1_sb = persist.tile([1, 512], BF16, tag="ones1", name="ones1")
    nc.vector.memset(ones1_sb[:], 1.0)
    warm = ps_s.tile([128, 512], F32, tag="s", name="ps_warm")
    for _ in range(6):
        nc.tensor.matmul(warm[:], ones1_sb[:, 0:128], ones1_sb[:],
                         start=True, stop=True)

    # ---- persistent SBUF tiles + input DMAs, emitted in consumer order ----
    # channel blocks live in a middle free dim so each tensor loads in one
    # (or a few) large DMAs instead of 4x4 small ones
    w2 = ptile([128, 4, 3 * C], BF16, "w2")
    xt2 = ptile([128, 4, N], BF16, "xt2")
    xtq2 = ptile([128, 4, NQ], BF16, "xtq2")
    wcat_r = wcat_d.rearrange("(i p) c -> p i c", p=128)
    xt_r = xt_d.rearrange("(i p) n -> p i n", p=128)

    # DMA order follows consumption order: V (position-descending) needs
    # w-v + xt blk3 first; K projections need wk + k tables; then Q inputs;
    # then bias patterns (jt-descending, slot1 first); proj weights last.
    for i in range(4):  # first consumers: small DMAs for fast first arrival
        nc.sync.dma_start(w2[:, i, 2 * C:3 * C], wcat_r[:, i, 2 * C:3 * C])
        nc.sync.dma_start(xt2[:, i, 1536:2048], xt_r[:, i, 1536:2048])
    for blk in (2, 1, 0):  # V consumes position blocks descending
        nc.sync.dma_start(xt2[:, :, blk * 512:(blk + 1) * 512],
                          xt_r[:, :, blk * 512:(blk + 1) * 512])
    nc.sync.dma_start(w2[:, :, C:2 * C], wcat_r[:, :, C:2 * C])
    cos2k = ptile([128, N], BF16, "cos2k")
    nc.sync.dma_start(cos2k[:], cos2k_d[:])
    sin2k = ptile([128, N], BF16, "sin2k")
    nc.sync.dma_start(sin2k[:], sin2k_d[:])
    rotperm_sb = ptile([128, 128], BF16, "rotperm")
    nc.sync.dma_start(rotperm_sb[:], rotperm_d[:])
    cos2q = ptile([128, NQ], BF16, "cos2q")
    nc.sync.dma_start(cos2q[:], cos2q_d[:])
    sin2q = ptile([128, NQ], BF16, "sin2q")
    nc.sync.dma_start(sin2q[:], sin2q_d[:])
    nc.sync.dma_start(w2[:, :, 0:C], wcat_r[:, :, 0:C])
    nc.sync.dma_start(xtq2[:, :, :],
                      xtq_d.rearrange("(i p) n -> p i n", p=128))
    ident_sb = ptile([128, 128], BF16, "ident128")
    nc.sync.dma_start(ident_sb[:], ident_d[:])

    c8eye2 = ptile([128, H, 128], BF16, "c8eye2")
    nc.sync.dma_start(c8eye2[:], c8eye_d.rearrange("h p e -> p h e"))
    pats1_t = ptile([128, 16, 512], BF16, "pats1t")
    pats1_r = pats1_d.rearrange("j p q -> p j q")
    for j0 in (12, 8, 4, 0):
        nc.sync.dma_start(pats1_t[:, j0:j0 + 4, :], pats1_r[:, j0:j0 + 4, :])
    pats0_t = ptile([128, 8, 512], BF16, "pats0t")
    pats0_r = pats0_d.rearrange("j p q -> p j q")
    for j0 in (4, 0):
        nc.sync.dma_start(pats0_t[:, j0:j0 + 4, :], pats0_r[:, j0:j0 + 4, :])
    projw4 = ptile([128, 4, 512], BF16, "projw4")
    nc.sync.dma_start(projw4[:], projwt_d.rearrange("(t p) c -> p t c", p=128))
    biasrow_sb = ptile([1, 512], BF16, "biasrow")
    nc.sync.dma_start(biasrow_sb[:], biasrow_d[:])

    q2_sb = [ptile([128, NQ], BF16, f"q2_{t}") for t in range(4)]
    k2_sb = [ptile([128, N], BF16, f"k2_{t}") for t in range(4)]
    v_sb = [ptile([128, H * 65], BF16, f"v_{nt}") for nt in range(JT)]
    # transposed attention tiles [dims(2 heads x 64), 128 queries] per (t, qg)
    outT = [[ptile([128, 128], BF16, f"oT{t}_{qg}") for qg in range(8)]
            for t in range(4)]

    # ---- V projection, position-descending (av streams consume jt=15 first)
    def emit_v(nt):
        # alternate V psums between the (idle) score pool and ps_a: more
        # slots in flight -> the psum ring is no longer copy-latency bound
        pool = ps_s if nt % 2 == 0 else ps_a
        tag = "s" if nt % 2 == 0 else "a"
        psv = pool.tile([128, 512], F32, tag=tag, name="ps_v")
        for ci in range(4):
            nc.tensor.matmul(
                psv[:], xt2[:, ci, nt * 128:(nt + 1) * 128],
                w2[:, ci, 2 * C:3 * C],
                start=(ci == 0), stop=(ci == 3))
        vdst = v_sb[nt].rearrange("p (h e) -> p h e", e=65)
        # DVE is the prephase pacer and ACT is idle there: alternate the
        # psum->sbuf copies between them
        if nt % 2 == 0:
            nc.vector.tensor_copy(vdst[:, :, 0:64],
                                   psv.rearrange("p (h e) -> p h e", e=64))
        else:
            nc.scalar.activation(vdst[:, :, 0:64],
                                 psv.rearrange("p (h e) -> p h e", e=64),
                                 mybir.ActivationFunctionType.Copy)
        nc.gpsimd.memset(vdst[:, :, 64:65], 1.0)

    # ---- Q/K projections + RoPE (psums from ps_a: no contention with the
    # score psums when chunks are injected into the score phase) ----
    # QK chunk, software-pipelined in two parts so the rotate matmul never
    # waits in-order on its own chunk's psum->sbuf copy.
    qk_pend = []

    def qk_part2(st):
        qsb, cos_sb, sin_sb, dst_sb, c0 = st
        ps_r = ps_a.tile([128, 512], F32, tag="a", name="ps_r")
        nc.tensor.matmul(ps_r[:], rotperm_sb[:], qsb[:], start=True, stop=True)
        tc_c = tmp_pool.tile([128, 512], BF16, tag="rt", name="rt_c")
        nc.vector.tensor_mul(tc_c[:], qsb[:], cos_sb[:, c0:c0 + 512])
        tc_u = tmp_pool.tile([128, 512], F32, tag="ru", name="rt_u")
        nc.vector.tensor_mul(tc_u[:], ps_r[:], sin_sb[:, c0:c0 + 512])
        nc.gpsimd.tensor_add(dst_sb[:, c0:c0 + 512], tc_c[:], tc_u[:])

    def qk_flush():
        while qk_pend:
            qk_part2(qk_pend.pop(0))

    def qk_chunk(dt_tile, w_off, rhs_sb, cos_sb, sin_sb, dst_sb, c0,
                 act_copy=False, defer=False):
        # one projection matmul set; rotate_half applied afterwards as a
        # +-1 permutation matmul (replaces the rotated-weight projection)
        ps_q = ps_a.tile([128, 512], F32, tag="a", name="ps_q")
        for ci in range(4):
            nc.tensor.matmul(
                ps_q[:],
                w2[:, ci, w_off + dt_tile * 128: w_off + (dt_tile + 1) * 128],
                rhs_sb[:, ci, c0:c0 + 512],
                start=(ci == 0), stop=(ci == 3))
        qsb = tmp_pool.tile([128, 512], BF16, tag="qs", name="qsb")
        if act_copy:  # ACT is idle in the upfront QK phase; DVE is the pacer
            nc.scalar.activation(qsb[:], ps_q[:],
                                 mybir.ActivationFunctionType.Copy)
        else:
            nc.vector.tensor_copy(qsb[:], ps_q[:])
        if defer:  # prephase: rot matmul runs under the next chunk's cover
            qk_pend.append((qsb, cos_sb, sin_sb, dst_sb, c0))
            if len(qk_pend) > 1:
                qk_part2(qk_pend.pop(0))
        else:
            qk_part2((qsb, cos_sb, sin_sb, dst_sb, c0))

    def qk_fillers(t, act_copy=False, defer=False):
        """QK work for head pair t as thunks: k chunks (desc), then q."""
        thunks = []
        for ch in range(N // 512 - 1, -1, -1):
            thunks.append(lambda ch=ch: qk_chunk(
                t, C, xt2, cos2k, sin2k, k2_sb[t], ch * 512, act_copy, defer))
        for ch in range(NQ // 512):
            thunks.append(lambda ch=ch: qk_chunk(
                t, 0, xtq2, cos2q, sin2q, q2_sb[t], ch * 512, act_copy, defer))
        return thunks

    # Interleave QK(t0) chunks with the V tail so the PE rides out the
    # later xt-block DMAs instead of stalling on them.
    qk0 = qk_fillers(0, act_copy=True, defer=True)  # k desc, then q
    for nt in range(15, 7, -1):
        emit_v(nt)
    qk0.pop(0)()
    qk0.pop(0)()
    for nt in range(7, 3, -1):
        emit_v(nt)
    qk0.pop(0)()
    for nt in range(3, -1, -1):
        emit_v(nt)
    for th in qk0:
        th()
    qk_flush()

    # ---- scores + ALiBi bias + exp + attn@v (stationary-exp orientation) ----
    # The two 512-query slots of a head pair run as interleaved streams so
    # the PE always has the other stream's score matmuls to chew on while
    # ACT computes this stream's exp (in-order PE would otherwise stall on
    # every av group).
    class Stream:
        def __init__(self, t, slot):
            self.t, self.slot = t, slot
            self.jts = [jt for jt in range(JT - 1, -1, -1)
                        if QHI[t][slot][jt] > 0]
            self.pats = pats1_t if slot == 1 else pats0_t
            self.avp = [ps_a.tile([128, 4 * 65], F32, tag="a",
                                  name=f"av{t}_{slot}_{p}") for p in range(2)]
            self.ks = 0          # score tiles emitted
            self.ka = 0          # av groups emitted
            self.pending = []    # (jt, qhi, et) awaiting av emission
            self.normed = False

        def scores_left(self):
            return self.ks < len(self.jts)

        def drain_one(self):
            if self.pending:
                self.emit_av()
            if self.ka == len(self.jts) and not self.normed:
                self.normed = True
                self.emit_normalize()

        def drain_all(self):
            while self.pending:
                self.drain_one()
            self.drain_one()

        def emit_scores(self):
            t, slot = self.t, self.slot
            jt = self.jts[self.ks]
            qhi = QHI[t][slot][jt]
            qlo = min(QLO[slot][jt], qhi)
            ps = ps_s.tile([128, 1024], F32, tag="s", name="ps_sc")
            for p in range(2):
                h = 2 * t + p
                ks = k2_sb[t][64 * p:64 * (p + 1), jt * 128:(jt + 1) * 128]
                qbase = slot * 512
                # One psum "zero region" (bank) per head-half: the first
                # matmul starts (lazy-zeroes) it, the last one stops it.
                if qlo >= qhi:  # no biased columns in this tile
                    nc.tensor.matmul(
                        ps[:, p * 512:p * 512 + qhi], ks,
                        q2_sb[t][64 * p:64 * (p + 1), qbase:qbase + qhi],
                        start=True, stop=True, tile_position=(64 * p, 0))
                else:
                    if qlo > 0:
                        nc.tensor.matmul(
                            ps[:, p * 512:p * 512 + qlo], ks,
                            q2_sb[t][64 * p:64 * (p + 1), qbase:qbase + qlo],
                            start=True, stop=False, tile_position=(64 * p, 0))
                    nc.tensor.matmul(
                        ps[:, p * 512 + qlo:p * 512 + qhi], ks,
                        q2_sb[t][64 * p:64 * (p + 1), qbase + qlo:qbase + qhi],
                        start=(qlo == 0), stop=False, tile_position=(64 * p, 0))
                    nc.tensor.matmul(
                        ps[:, p * 512 + qlo:p * 512 + qhi], c8eye2[:, h, :],
                        self.pats[:, jt, qlo:qhi],
                        start=False, stop=True, tile_position=(0, 0))
            et = exp_pool.tile([128, 1024], BF16, tag="e", name="et")
            if qhi == 512:
                nc.scalar.activation(et[:], ps[:], Exp)
            else:
                psv_ap = ps.rearrange("p (h q) -> p h q", h=2)[:, :, 0:qhi]
                etv_ap = et.rearrange("p (h q) -> p h q", h=2)[:, :, 0:qhi]
                nc.scalar.activation(etv_ap, psv_ap, Exp)
            self.pending.append((jt, qhi, et))
            self.ks += 1

        def emit_av(self):
            t, slot = self.t, self.slot
            jt, qhi, et = self.pending.pop(0)
            first = (self.ka == 0)
            jt_last = self.jts[-1]
            for p in range(2):
                h = 2 * t + p
                vs = v_sb[jt][:, h * 65:(h + 1) * 65]
                for qg in range(4):
                    qw = min(128, qhi - 128 * qg)
                    if qw <= 0:
                        break
                    is_last = (jt == jt_last
                               and (qw < 128 or qg == 3
                                    or qhi - 128 * (qg + 1) <= 0))
                    nc.tensor.matmul(
                        self.avp[p][0:qw, qg * 65:(qg + 1) * 65],
                        et[:, p * 512 + qg * 128:p * 512 + qg * 128 + qw],
                        vs,
                        start=(first and qg == 0),
                        stop=is_last,
                        skip_group_check=True)
            self.ka += 1

        def emit_normalize(self):
            t, slot = self.t, self.slot
            recs = []
            for p in range(2):
                rec = rec_pool.tile([128, 4], F32, tag="r", name="rec")
                nc.vector.reciprocal(
                    rec[:],
                    self.avp[p].rearrange("p (g e) -> p g e", e=65)[:, :, 64])
                recs.append(rec)
            for qg in range(4):
                att = att_pool.tile([128, 128], BF16, tag="t", name="att")
                for p in range(2):
                    nc.vector.tensor_scalar_mul(
                        att[:, p * 64:(p + 1) * 64],
                        self.avp[p][:, qg * 65:qg * 65 + 64],
                        recs[p][:, qg:qg + 1])
                if t == 3 and slot == 1:
                    # tail: PE transpose (+DVE copy) beats the serialized
                    # HWDGE xbar path on the critical path to the projection
                    ps_t = ps_a.tile([128, 128], BF16, tag="a", name="ps_t")
                    nc.tensor.transpose(ps_t[:], att[:], ident_sb[:])
                    nc.vector.tensor_copy(outT[t][slot * 4 + qg][:], ps_t[:])
                else:
                    nc.sync.dma_start_transpose(outT[t][slot * 4 + qg][:],
                                                att[:])

    def emit_proj(qg):
        psp = ps_a.tile([128, 512], F32, tag="a", name="ps_proj")
        for tt in range(4):
            nc.tensor.matmul(psp[:], outT[tt][qg][:], projw4[:, tt, :],
                             start=(tt == 0), stop=False)
        nc.tensor.matmul(psp[:], ones1_sb[:, 0:128], biasrow_sb[:],
                         start=False, stop=True)
        fin = fin_pool.tile([128, 512], F32, tag="f", name="fin")
        nc.scalar.activation(fin[:], psp[:], mybir.ActivationFunctionType.Copy)
        nc.sync.dma_start(out_d[qg * 128:(qg + 1) * 128, :], fin[:])

    prev_stream = None
    for t in range(4):
        # Fillers injected into score-phase PE slack (they use ps_a slots,
        # not the score psums): QK chunks of the next head pair, and for the
        # last pair's slot1 the first half of the output projection (whose
        # outT inputs - slot0 of every pair - are complete by then).
        for slot in range(2):
            if t < 3:
                fillers = qk_fillers(t + 1) if slot == 0 else fillers
            else:
                fillers = [] if slot == 0 else [
                    (lambda qg=qg: emit_proj(qg)) for qg in range(4)]
            n_tiles = len([jt for jt in range(JT) if QHI[t][slot][jt] > 0])
            inject_every = max(3, n_tiles // max(1, len(fillers) or 1))
            tiles_done = 0
            s = Stream(t, slot)
            # prologue: two tiles in flight, then drain the previous
            # stream's tail under their cover (hides its last exp latency)
            s.emit_scores()
            if s.scores_left():
                s.emit_scores()
            if prev_stream is not None:
                prev_stream.drain_all()
            tiles_done = s.ks
            while s.scores_left():
                s.emit_scores()
                while len(s.pending) > 5:  # five-deep exp/av stagger
                    s.drain_one()
                tiles_done += 1
                if (fillers and tiles_done % inject_every == 0
                        and tiles_done <= n_tiles - 2):
                    fillers.pop(0)()
            while len(s.pending) > 2:
                s.drain_one()
            prev_stream = s
            if t == 3 or slot == 1:
                for th in fillers:
                    th()
                fillers = []
                qk_flush()
    prev_stream.drain_all()

    # ---- second half of the output projection (slot1 positions) ----
    # pairs share one staging tile and one output DMA: fewer serialized
    # HWDGE windows on the final critical path
    for qg in (4,):
        fin2 = fin_pool.tile([128, 2, 512], F32, tag="f2", name="fin2")
        for sub in range(2):
            psp = ps_s.tile([128, 512], F32, tag="s", name="ps_proj")
            for tt in range(4):
                nc.tensor.matmul(psp[:], outT[tt][qg + sub][:],
                                 projw4[:, tt, :], start=(tt == 0), stop=False)
            nc.tensor.matmul(psp[:], ones1_sb[:, 0:128], biasrow_sb[:],
                             start=False, stop=True)
            nc.scalar.activation(fin2[:, sub], psp[:],
                                 mybir.ActivationFunctionType.Copy)
        nc.sync.dma_start(
            out_d[qg * 128:(qg + 2) * 128, :].rearrange(
                "(s p) c -> p s c", p=128), fin2[:])
    # last two outputs unpaired: their single-width DMAs overlap the copies
    # instead of waiting for both, shortening the final critical chain
    emit_proj(6)
    emit_proj(7)

    ctx.close()


@functools.lru_cache(maxsize=1)
def _graph():
    return _build_graph()


def kernel(x, qkv_w, proj_w, proj_b):
    global LAST_RESULT
    x = np.asarray(x, np.float32)
    qkv_w = np.asarray(qkv_w, np.float32)
    proj_w = np.asarray(proj_w, np.float32)
    proj_b = np.asarray(proj_b, np.float32)

    nc = _graph()
    shared, sin, cos = _shared_inputs(qkv_w, proj_w, proj_b)
    in_maps = [_core_inputs(c, x, shared, sin, cos) for c in range(NCORES)]
    trace = bool(int(os.environ.get("KERNEL_TRACE", "0")))
    res = bass_utils.run_bass_kernel_spmd(nc, in_maps,
                                          core_ids=list(range(NCORES)),
                                          trace=trace)
    LAST_RESULT = res
    out = np.zeros((B, N, C), np.float32)
    for c in range(NCORES):
        b, s = c // 2, c % 2
        blocks = _owned_blocks(s)
        o = np.asarray(res.results[c]["out"], np.float32)
        out[b, blocks[0] * 512:(blocks[0] + 1) * 512] = o[0:512]
        out[b, blocks[1] * 512:(blocks[1] + 1) * 512] = o[512:1024]
    return out


# revision 88
# speedup vs baseline: 1.0049x; 1.0049x over previous
"""Fused multi-head attention layer (RoPE + ALiBi + softmax + out-proj) on 8 TRN2 cores.

Sharding: core c -> (batch b = c//2, query-half s = c%2). Each core owns 1024
queries of its batch (two 512-blocks, interleaved for ALiBi load balance),
computes K/V for all 2048 positions, and writes a disjoint slice of the output.
No collectives. All 8 cores run one SPMD graph; per-core differences (which
query blocks, ALiBi band offsets) are encoded purely in host-prepared data.

v2 optimizations (vs baseline, TimelineSim 222.6us -> 155.4us):
- Score/bias/exp/AV column-trimmed to the ALiBi-needed query prefix per
  (head-pair, key-tile, slot); bias matmuls further restricted to the
  biased suffix via split accumulation groups; av matmuls additionally
  trimmed to each head's own radius (dropped columns hold exp(<-30)).
- Both heads of a pair share one [128,1024] score PSUM -> one exp
  activation per (pair, jt, slot), halving ACT fixed overhead.
- AV uses stationary=exp-tile / moving=v (65 cols) instead of streaming
  512 query columns: ~2x less PE stream time; output lands [query, dim].
- Softmax normalization becomes a per-partition tensor_scalar multiply.
- Attention tiles transposed on the DMA engines (xbar), projection runs
  with 128-deep contraction (head pairs packed), proj bias folded in as a
  rank-1 ones-row matmul.
- RoPE rotate_half applied as a +-1 permutation matmul on the projected
  q/k instead of a second rotated-weight projection (drops 96 projection
  matmuls); the psum->sbuf hop also makes the cos-multiply all-bf16.
- Deep software pipelining: score streams run one (t, slot) at a time
  with a five-deep exp/av stagger, chained across stream boundaries; QK
  chunks of the next head pair and the first half of the projection are
  injected as fillers into score-phase PE slack (on the ps_a pool, so
  they never contend with score psums).
- Inputs land in a few large multi-dim DMAs ordered by first consumer;
  the tail transposes bypass the DMA queue via PE transpose; V-phase
  psums alternate between both psum pools so the ring is never
  copy-latency bound; the final output DMAs are paired to halve the
  serialized HWDGE windows on the tail; a throwaway warmup matmul chain
  starts the PE p-state ramp clock early so real matmuls never pay the
  cold-clock penalty; prephase psum->sbuf copies split across DVE/ACT
  and the prephase rotate matmuls deferred one chunk.

(fp8 DoubleRow for the bias matmul halves its PE cost in the cost model
and passes CoreSim numerically, but the axon PJRT execution path fails
on it at runtime, so it stays bf16.)
"""

import functools
import os
import sys

import numpy as np

sys.path.insert(0, "/opt/trn_rl_repo")

import ml_dtypes  # noqa: E402

import concourse.bass as bass  # noqa: E402
import concourse.tile as tile  # noqa: E402
from concourse import bacc, mybir, bass_utils  # noqa: E402

BF16 = mybir.dt.bfloat16
F32 = mybir.dt.float32
NPBF = ml_dtypes.bfloat16

B, N, C, H, D = 4, 2048, 512, 8, 64
NCORES = 8
NQ = 1024            # local queries per core
JT = N // 128        # 16 j-tiles of 128 key positions
T_CUT = 30.0         # ALiBi cutoff in logits: exp(-30) is negligible
SCALE = D ** -0.5

# c8_h = alibi_slope_h * MAX_BIAS = 2^-(h+1) * 8 = 2^(2-h)
C8 = [2.0 ** (2 - h) for h in range(H)]
# band reach (in key positions) per head
RADIUS = [T_CUT / c for c in C8]

# SPMD union bounds over the two cores sharing a slot index:
# slot0 owns blocks {0,1}*512, slot1 owns {2,3}*512.
I0MIN = [0, 1024]
I0MAX = [512, 1536]


def _qhi(t, slot, jt):
    """Needed query-column prefix of the [128 keys x 512 q] tile (pair union)."""
    return max(
        max(0, min(512, 128 * jt + 127 + int(RADIUS[h]) + 1 - I0MIN[slot]))
        for h in (2 * t, 2 * t + 1))


def _qlo(slot, jt):
    """First query column where ALiBi bias can be nonzero (union over cores)."""
    return max(0, min(512, 128 * jt + 1 - I0MAX[slot]))


QHI = [[[_qhi(t, s, jt) for jt in range(JT)] for s in range(2)] for t in range(4)]
QLO = [[_qlo(s, jt) for jt in range(JT)] for s in range(2)]
# per (t, slot, qg): last (smallest) jt in descending order that writes qg
JSTOP = [[[min(jt for jt in range(JT) if QHI[t][s][jt] > 128 * qg)
           for qg in range(4)] for s in range(2)] for t in range(4)]

# per-head widths: av matmuls beyond a head's own radius act on exp(<-30)=~0
# columns and are dropped (contribution ~1e-13; scores/bias/exp keep the
# pair width so the psum/activation structure is untouched)
QHIH = [[[[max(0, min(512, 128 * jt + 127 + int(RADIUS[2 * t + p]) + 1
                      - I0MIN[s])) for p in range(2)] for jt in range(JT)]
         for s in range(2)] for t in range(4)]
JSTOPH = [[[[min(jt for jt in range(JT) if QHIH[t][s][jt][p] > 128 * qg)
             for qg in range(4)] for p in range(2)] for s in range(2)]
          for t in range(4)]

LAST_RESULT = None  # test harness reads exec_time_ns from here


def _owned_blocks(s):
    # 512-query blocks of the batch owned by query-half s (balanced for ALiBi)
    return (0, 3) if s == 0 else (1, 2)


def _rope_tables():
    inv = 1.0 / (10000.0 ** (np.arange(0, D, 2, dtype=np.float32) / D))
    f = np.arange(N, dtype=np.float32)[:, None] * inv[None, :]
    sin = np.concatenate([np.sin(f), np.sin(f)], axis=-1).astype(np.float32)
    cos = np.concatenate([np.cos(f), np.cos(f)], axis=-1).astype(np.float32)
    return sin, cos  # [N, D]


def _shared_inputs(qkv_w, proj_w, proj_b):
    wqT = np.ascontiguousarray(qkv_w[0:C].T) * SCALE       # [C, C]
    wkT = np.ascontiguousarray(qkv_w[C:2 * C].T)
    wvT = np.ascontiguousarray(qkv_w[2 * C:3 * C].T)
    wcat = np.concatenate([wqT, wkT, wvT], axis=1).astype(NPBF)

    ident128 = np.eye(128, dtype=np.float32)

    # rotate_half as a +-1 permutation: out[i] = -in[32+i], out[32+i] = in[i]
    # per 64-dim head; lhsT layout [d_in, d_out].
    rotperm = np.zeros((128, 128), np.float32)
    for hh in range(2):
        for i in range(32):
            rotperm[hh * 64 + 32 + i, hh * 64 + i] = -1.0
            rotperm[hh * 64 + i, hh * 64 + 32 + i] = 1.0

    c8eye = np.zeros((H, 128, 128), np.float32)
    for h in range(H):
        np.fill_diagonal(c8eye[h], C8[h])

    sin, cos = _rope_tables()
    cos2k = np.tile(cos.T, (2, 1))                         # [128, N]
    sin2k = np.tile(sin.T, (2, 1))
    return {
        "wcat": wcat,
        "rotperm": rotperm.astype(NPBF),
        "ident128": ident128.astype(NPBF),
        "c8eye": c8eye.astype(NPBF),
        "projwt": np.ascontiguousarray(proj_w.T).astype(NPBF),
        "biasrow": proj_b[None, :].astype(NPBF),
        "cos2k": cos2k.astype(NPBF), "sin2k": sin2k.astype(NPBF),
    }, sin, cos


def _pats_for(i0):
    jl = np.arange(128, dtype=np.float32)[:, None]
    il = np.arange(512, dtype=np.float32)[None, :]
    return [np.minimum((jt * 128 + jl) - (i0 + il), 0.0).astype(NPBF)
            for jt in range(16)]


def _core_inputs(c, x, shared, sin, cos):
    b, s = c // 2, c % 2
    blocks = _owned_blocks(s)
    gi = np.concatenate([np.arange(blk * 512, (blk + 1) * 512) for blk in blocks])

    xt = np.ascontiguousarray(x[b].T)                      # [C, N]
    xtq = np.ascontiguousarray(x[b][gi].T)                 # [C, NQ]

    cos2q = np.tile(cos[gi].T, (2, 1))                     # [128, NQ]
    sin2q = np.tile(sin[gi].T, (2, 1))

    pats0 = np.stack(_pats_for(blocks[0] * 512)[:8])
    pats1 = np.stack(_pats_for(blocks[1] * 512))

    return {
        "xt": xt.astype(NPBF),
        "xtq": xtq.astype(NPBF),
        "cos2q": cos2q.astype(NPBF), "sin2q": sin2q.astype(NPBF),
        "pats0": pats0,
        "pats1": pats1,
        **shared,
    }


def _build_graph():
    nc = bacc.Bacc("TRN2", target_bir_lowering=False, debug=False,
                   num_devices=NCORES)

    xt_d = nc.dram_tensor("xt", [C, N], BF16, kind="ExternalInput").ap()
    xtq_d = nc.dram_tensor("xtq", [C, NQ], BF16, kind="ExternalInput").ap()
    wcat_d = nc.dram_tensor("wcat", [C, 3 * C], BF16, kind="ExternalInput").ap()
    rotperm_d = nc.dram_tensor("rotperm", [128, 128], BF16, kind="ExternalInput").ap()
    ident_d = nc.dram_tensor("ident128", [128, 128], BF16, kind="ExternalInput").ap()
    cos2q_d = nc.dram_tensor("cos2q", [128, NQ], BF16, kind="ExternalInput").ap()
    sin2q_d = nc.dram_tensor("sin2q", [128, NQ], BF16, kind="ExternalInput").ap()
    cos2k_d = nc.dram_tensor("cos2k", [128, N], BF16, kind="ExternalInput").ap()
    sin2k_d = nc.dram_tensor("sin2k", [128, N], BF16, kind="ExternalInput").ap()
    pats0_d = nc.dram_tensor("pats0", [8, 128, 512], BF16, kind="ExternalInput").ap()
    pats1_d = nc.dram_tensor("pats1", [16, 128, 512], BF16, kind="ExternalInput").ap()
    c8eye_d = nc.dram_tensor("c8eye", [H, 128, 128], BF16, kind="ExternalInput").ap()
    projwt_d = nc.dram_tensor("projwt", [C, C], BF16, kind="ExternalInput").ap()
    biasrow_d = nc.dram_tensor("biasrow", [1, 512], BF16, kind="ExternalInput").ap()
    out_d = nc.dram_tensor("out", [NQ, C], F32, kind="ExternalOutput").ap()

    with tile.TileContext(nc) as tc:
        _body(nc, tc, xt_d, xtq_d, wcat_d, rotperm_d, ident_d, cos2q_d,
              sin2q_d, cos2k_d, sin2k_d, pats0_d, pats1_d, c8eye_d, projwt_d,
              biasrow_d, out_d)
    nc.compile()
    return nc


def _body(nc, tc, xt_d, xtq_d, wcat_d, rotperm_d, ident_d, cos2q_d,
          sin2q_d, cos2k_d, sin2k_d, pats0_d, pats1_d, c8eye_d, projwt_d,
          biasrow_d, out_d):
    from contextlib import ExitStack
    ctx = ExitStack()
    persist = ctx.enter_context(tc.tile_pool(name="persist", bufs=1))
    tmp_pool = ctx.enter_context(tc.tile_pool(name="ropetmp", bufs=6))
    exp_pool = ctx.enter_context(tc.tile_pool(name="exp", bufs=10))
    fin_pool = ctx.enter_context(tc.tile_pool(name="final", bufs=2))
    att_pool = ctx.enter_context(tc.tile_pool(name="att", bufs=8))
    rec_pool = ctx.enter_context(tc.tile_pool(name="rec", bufs=4))
    # PSUM: ps_s = 2 bufs x [128,1024] f32 (2 banks each); ps_a = 4 bufs x
    # [128,512] f32 (1 bank each) shared by QKV-phase psums and AV accums.
    ps_s = ctx.enter_context(tc.tile_pool(name="ps_s", bufs=2, space="PSUM"))
    ps_a = ctx.enter_context(tc.tile_pool(name="ps_a", bufs=4, space="PSUM"))

    def ptile(shape, dtype, tag):
        return persist.tile(shape, dtype, tag=tag, name=tag)

    Exp = mybir.ActivationFunctionType.Exp

    # PE p-state warmup: the cost model ramps 0.65 -> 1.2 -> 2.4 GHz over
    # ~3us of continuous execution. A chain of throwaway matmuls (dependent
    # only on an early memset) starts the ramp clock at ~0.3us so the first
    # real V matmuls already run at full clock.
    ones1_sb = persist.tile([1, 512], BF16, tag="ones1", name="ones1")
    nc.vector.memset(ones1_sb[:], 1.0)
    warm = ps_s.tile([128, 512], F32, tag="s", name="ps_warm")
    for _ in range(6):
        nc.tensor.matmul(warm[:], ones1_sb[:, 0:128], ones1_sb[:],
                         start=True, stop=True)

    # ---- persistent SBUF tiles + input DMAs, emitted in consumer order ----
    # channel blocks live in a middle free dim so each tensor loads in one
    # (or a few) large DMAs instead of 4x4 small ones
    w2 = ptile([128, 4, 3 * C], BF16, "w2")
    xt2 = ptile([128, 4, N], BF16, "xt2")
    xtq2 = ptile([128, 4, NQ], BF16, "xtq2")
    wcat_r = wcat_d.rearrange("(i p) c -> p i c", p=128)
    xt_r = xt_d.rearrange("(i p) n -> p i n", p=128)

    # DMA order follows consumption order: V (position-descending) needs
    # w-v + xt blk3 first; K projections need wk + k tables; then Q inputs;
    # then bias patterns (jt-descending, slot1 first); proj weights last.
    for i in range(4):  # first consumers: small DMAs for fast first arrival
        nc.sync.dma_start(w2[:, i, 2 * C:3 * C], wcat_r[:, i, 2 * C:3 * C])
        nc.sync.dma_start(xt2[:, i, 1536:2048], xt_r[:, i, 1536:2048])
    for blk in (2, 1, 0):  # V consumes position blocks descending
        nc.sync.dma_start(xt2[:, :, blk * 512:(blk + 1) * 512],
                          xt_r[:, :, blk * 512:(blk + 1) * 512])
    nc.sync.dma_start(w2[:, :, C:2 * C], wcat_r[:, :, C:2 * C])
    cos2k = ptile([128, N], BF16, "cos2k")
    nc.sync.dma_start(cos2k[:], cos2k_d[:])
    sin2k = ptile([128, N], BF16, "sin2k")
    nc.sync.dma_start(sin2k[:], sin2k_d[:])
    rotperm_sb = ptile([128, 128], BF16, "rotperm")
    nc.sync.dma_start(rotperm_sb[:], rotperm_d[:])
    cos2q = ptile([128, NQ], BF16, "cos2q")
    nc.sync.dma_start(cos2q[:], cos2q_d[:])
    sin2q = ptile([128, NQ], BF16, "sin2q")
    nc.sync.dma_start(sin2q[:], sin2q_d[:])
    nc.sync.dma_start(w2[:, :, 0:C], wcat_r[:, :, 0:C])
    nc.sync.dma_start(xtq2[:, :, :],
                      xtq_d.rearrange("(i p) n -> p i n", p=128))
    ident_sb = ptile([128, 128], BF16, "ident128")
    nc.sync.dma_start(ident_sb[:], ident_d[:])

    c8eye2 = ptile([128, H, 128], BF16, "c8eye2")
    nc.sync.dma_start(c8eye2[:], c8eye_d.rearrange("h p e -> p h e"))
    pats1_t = ptile([128, 16, 512], BF16, "pats1t")
    pats1_r = pats1_d.rearrange("j p q -> p j q")
    for j0 in (12, 8, 4, 0):
        nc.sync.dma_start(pats1_t[:, j0:j0 + 4, :], pats1_r[:, j0:j0 + 4, :])
    pats0_t = ptile([128, 8, 512], BF16, "pats0t")
    pats0_r = pats0_d.rearrange("j p q -> p j q")
    for j0 in (4, 0):
        nc.sync.dma_start(pats0_t[:, j0:j0 + 4, :], pats0_r[:, j0:j0 + 4, :])
    projw4 = ptile([128, 4, 512], BF16, "projw4")
    nc.sync.dma_start(projw4[:], projwt_d.rearrange("(t p) c -> p t c", p=128))
    biasrow_sb = ptile([1, 512], BF16, "biasrow")
    nc.sync.dma_start(biasrow_sb[:], biasrow_d[:])

    q2_sb = [ptile([128, NQ], BF16, f"q2_{t}") for t in range(4)]
    k2_sb = [ptile([128, N], BF16, f"k2_{t}") for t in range(4)]
    v_sb = [ptile([128, H * 65], BF16, f"v_{nt}") for nt in range(JT)]
    # transposed attention tiles [dims(2 heads x 64), 128 queries] per (t, qg)
    outT = [[ptile([128, 128], BF16, f"oT{t}_{qg}") for qg in range(8)]
            for t in range(4)]

    # ---- V projection, position-descending (av streams consume jt=15 first)
    def emit_v(nt):
        # alternate V psums between the (idle) score pool and ps_a: more
        # slots in flight -> the psum ring is no longer copy-latency bound
        pool = ps_s if nt % 2 == 0 else ps_a
        tag = "s" if nt % 2 == 0 else "a"
        psv = pool.tile([128, 512], F32, tag=tag, name="ps_v")
        for ci in range(4):
            nc.tensor.matmul(
                psv[:], xt2[:, ci, nt * 128:(nt + 1) * 128],
                w2[:, ci, 2 * C:3 * C],
                start=(ci == 0), stop=(ci == 3))
        vdst = v_sb[nt].rearrange("p (h e) -> p h e", e=65)
        # DVE is the prephase pacer and ACT is idle there: alternate the
        # psum->sbuf copies between them
        if nt % 2 == 0:
            nc.vector.tensor_copy(vdst[:, :, 0:64],
                                   psv.rearrange("p (h e) -> p h e", e=64))
        else:
            nc.scalar.activation(vdst[:, :, 0:64],
                                 psv.rearrange("p (h e) -> p h e", e=64),
                                 mybir.ActivationFunctionType.Copy)
        nc.gpsimd.memset(vdst[:, :, 64:65], 1.0)

    # ---- Q/K projections + RoPE (psums from ps_a: no contention with the
    # score psums when chunks are injected into the score phase) ----
    # QK chunk, software-pipelined in two parts so the rotate matmul never
    # waits in-order on its own chunk's psum->sbuf copy.
    qk_pend = []

    def qk_part2(st):
        qsb, cos_sb, sin_sb, dst_sb, c0 = st
        ps_r = ps_a.tile([128, 512], F32, tag="a", name="ps_r")
        nc.tensor.matmul(ps_r[:], rotperm_sb[:], qsb[:], start=True, stop=True)
        tc_c = tmp_pool.tile([128, 512], BF16, tag="rt", name="rt_c")
        nc.vector.tensor_mul(tc_c[:], qsb[:], cos_sb[:, c0:c0 + 512])
        tc_u = tmp_pool.tile([128, 512], F32, tag="ru", name="rt_u")
        nc.vector.tensor_mul(tc_u[:], ps_r[:], sin_sb[:, c0:c0 + 512])
        nc.gpsimd.tensor_add(dst_sb[:, c0:c0 + 512], tc_c[:], tc_u[:])

    def qk_flush():
        while qk_pend:
            qk_part2(qk_pend.pop(0))

    def qk_chunk(dt_tile, w_off, rhs_sb, cos_sb, sin_sb, dst_sb, c0,
                 act_copy=False, defer=False):
        # one projection matmul set; rotate_half applied afterwards as a
        # +-1 permutation matmul (replaces the rotated-weight projection)
        ps_q = ps_a.tile([128, 512], F32, tag="a", name="ps_q")
        for ci in range(4):
            nc.tensor.matmul(
                ps_q[:],
                w2[:, ci, w_off + dt_tile * 128: w_off + (dt_tile + 1) * 128],
                rhs_sb[:, ci, c0:c0 + 512],
                start=(ci == 0), stop=(ci == 3))
        qsb = tmp_pool.tile([128, 512], BF16, tag="qs", name="qsb")
        if act_copy:  # ACT is idle in the upfront QK phase; DVE is the pacer
            nc.scalar.activation(qsb[:], ps_q[:],
                                 mybir.ActivationFunctionType.Copy)
        else:
            nc.vector.tensor_copy(qsb[:], ps_q[:])
        if defer:  # prephase: rot matmul runs under the next chunk's cover
            qk_pend.append((qsb, cos_sb, sin_sb, dst_sb, c0))
            if len(qk_pend) > 1:
                qk_part2(qk_pend.pop(0))
        else:
            qk_part2((qsb, cos_sb, sin_sb, dst_sb, c0))

    def qk_fillers(t, act_copy=False, defer=False):
        """QK work for head pair t as thunks: k chunks (desc), then q."""
        thunks = []
        for ch in range(N // 512 - 1, -1, -1):
            thunks.append(lambda ch=ch: qk_chunk(
                t, C, xt2, cos2k, sin2k, k2_sb[t], ch * 512, act_copy, defer))
        for ch in range(NQ // 512):
            thunks.append(lambda ch=ch: qk_chunk(
                t, 0, xtq2, cos2q, sin2q, q2_sb[t], ch * 512, act_copy, defer))
        return thunks

    # Interleave QK(t0) chunks with the V tail so the PE rides out the
    # later xt-block DMAs instead of stalling on them.
    qk0 = qk_fillers(0, act_copy=True, defer=True)  # k desc, then q
    for nt in range(15, 7, -1):
        emit_v(nt)
    qk0.pop(0)()
    qk0.pop(0)()
    for nt in range(7, 3, -1):
        emit_v(nt)
    qk0.pop(0)()
    for nt in range(3, -1, -1):
        emit_v(nt)
    for th in qk0:
        th()
    qk_flush()

    # ---- scores + ALiBi bias + exp + attn@v (stationary-exp orientation) ----
    # The two 512-query slots of a head pair run as interleaved streams so
    # the PE always has the other stream's score matmuls to chew on while
    # ACT computes this stream's exp (in-order PE would otherwise stall on
    # every av group).
    class Stream:
        def __init__(self, t, slot):
            self.t, self.slot = t, slot
            self.jts = [jt for jt in range(JT - 1, -1, -1)
                        if QHI[t][slot][jt] > 0]
            self.pats = pats1_t if slot == 1 else pats0_t
            self.avp = [ps_a.tile([128, 4 * 65], F32, tag="a",
                                  name=f"av{t}_{slot}_{p}") for p in range(2)]
            self.ks = 0          # score tiles emitted
            self.ka = 0          # av groups emitted
            self.pending = []    # (jt, qhi, et) awaiting av emission
            self.normed = False

        def scores_left(self):
            return self.ks < len(self.jts)

        def drain_one(self):
            if self.pending:
                self.emit_av()
            if self.ka == len(self.jts) and not self.normed:
                self.normed = True
                self.emit_normalize()

        def drain_all(self):
            while self.pending:
                self.drain_one()
            self.drain_one()

        def emit_scores(self):
            t, slot = self.t, self.slot
            jt = self.jts[self.ks]
            qhi = QHI[t][slot][jt]
            qlo = min(QLO[slot][jt], qhi)
            ps = ps_s.tile([128, 1024], F32, tag="s", name="ps_sc")
            for p in range(2):
                h = 2 * t + p
                ks = k2_sb[t][64 * p:64 * (p + 1), jt * 128:(jt + 1) * 128]
                qbase = slot * 512
                # One psum "zero region" (bank) per head-half: the first
                # matmul starts (lazy-zeroes) it, the last one stops it.
                if qlo >= qhi:  # no biased columns in this tile
                    nc.tensor.matmul(
                        ps[:, p * 512:p * 512 + qhi], ks,
                        q2_sb[t][64 * p:64 * (p + 1), qbase:qbase + qhi],
                        start=True, stop=True, tile_position=(64 * p, 0))
                else:
                    if qlo > 0:
                        nc.tensor.matmul(
                            ps[:, p * 512:p * 512 + qlo], ks,
                            q2_sb[t][64 * p:64 * (p + 1), qbase:qbase + qlo],
                            start=True, stop=False, tile_position=(64 * p, 0))
                    nc.tensor.matmul(
                        ps[:, p * 512 + qlo:p * 512 + qhi], ks,
                        q2_sb[t][64 * p:64 * (p + 1), qbase + qlo:qbase + qhi],
                        start=(qlo == 0), stop=False, tile_position=(64 * p, 0))
                    nc.tensor.matmul(
                        ps[:, p * 512 + qlo:p * 512 + qhi], c8eye2[:, h, :],
                        self.pats[:, jt, qlo:qhi],
                        start=False, stop=True, tile_position=(0, 0))
            et = exp_pool.tile([128, 1024], BF16, tag="e", name="et")
            if qhi == 512:
                nc.scalar.activation(et[:], ps[:], Exp)
            else:
                psv_ap = ps.rearrange("p (h q) -> p h q", h=2)[:, :, 0:qhi]
                etv_ap = et.rearrange("p (h q) -> p h q", h=2)[:, :, 0:qhi]
                nc.scalar.activation(etv_ap, psv_ap, Exp)
            self.pending.append((jt, qhi, et))
            self.ks += 1

        def emit_av(self):
            t, slot = self.t, self.slot
            jt, qhi, et = self.pending.pop(0)
            first = (self.ka == 0)
            for p in range(2):
                h = 2 * t + p
                qhi_p = min(qhi, QHIH[t][slot][jt][p])
                vs = v_sb[jt][:, h * 65:(h + 1) * 65]
                for qg in range(4):
                    qw = min(128, qhi_p - 128 * qg)
                    if qw <= 0:
                        break
                    nc.tensor.matmul(
                        self.avp[p][0:qw, qg * 65:(qg + 1) * 65],
                        et[:, p * 512 + qg * 128:p * 512 + qg * 128 + qw],
                        vs,
                        start=(first and qg == 0),
                        stop=(jt == JSTOPH[t][slot][p][qg]),
                        skip_group_check=True)
            self.ka += 1

        def emit_normalize(self):
            t, slot = self.t, self.slot
            recs = []
            for p in range(2):
                rec = rec_pool.tile([128, 4], F32, tag="r", name="rec")
                nc.vector.reciprocal(
                    rec[:],
                    self.avp[p].rearrange("p (g e) -> p g e", e=65)[:, :, 64])
                recs.append(rec)
            for qg in range(4):
                att = att_pool.tile([128, 128], BF16, tag="t", name="att")
                for p in range(2):
                    nc.vector.tensor_scalar_mul(
                        att[:, p * 64:(p + 1) * 64],
                        self.avp[p][:, qg * 65:qg * 65 + 64],
                        recs[p][:, qg:qg + 1])
                if t == 3 and slot == 1:
                    # tail: PE transpose (+DVE copy) beats the serialized
                    # HWDGE xbar path on the critical path to the projection
                    ps_t = ps_a.tile([128, 128], BF16, tag="a", name="ps_t")
                    nc.tensor.transpose(ps_t[:], att[:], ident_sb[:])
                    nc.vector.tensor_copy(outT[t][slot * 4 + qg][:], ps_t[:])
                else:
                    nc.sync.dma_start_transpose(outT[t][slot * 4 + qg][:],
                                                att[:])

    def emit_proj(qg):
        psp = ps_a.tile([128, 512], F32, tag="a", name="ps_proj")
        for tt in range(4):
            nc.tensor.matmul(psp[:], outT[tt][qg][:], projw4[:, tt, :],
                             start=(tt == 0), stop=False)
        nc.tensor.matmul(psp[:], ones1_sb[:, 0:128], biasrow_sb[:],
                         start=False, stop=True)
        fin = fin_pool.tile([128, 512], F32, tag="f", name="fin")
        nc.scalar.activation(fin[:], psp[:], mybir.ActivationFunctionType.Copy)
        nc.sync.dma_start(out_d[qg * 128:(qg + 1) * 128, :], fin[:])

    prev_stream = None
    for t in range(4):
        # Fillers injected into score-phase PE slack (they use ps_a slots,
        # not the score psums): QK chunks of the next head pair, and for the
        # last pair's slot1 the first half of the output projection (whose
        # outT inputs - slot0 of every pair - are complete by then).
        for slot in range(2):
            if t < 3:
                fillers = qk_fillers(t + 1) if slot == 0 else fillers
            else:
                fillers = [] if slot == 0 else [
                    (lambda qg=qg: emit_proj(qg)) for qg in range(4)]
            n_tiles = len([jt for jt in range(JT) if QHI[t][slot][jt] > 0])
            inject_every = max(3, n_tiles // max(1, len(fillers) or 1))
            tiles_done = 0
            s = Stream(t, slot)
            # prologue: two tiles in flight, then drain the previous
            # stream's tail under their cover (hides its last exp latency)
            s.emit_scores()
            if s.scores_left():
                s.emit_scores()
            if prev_stream is not None:
                prev_stream.drain_all()
            tiles_done = s.ks
            while s.scores_left():
                s.emit_scores()
                while len(s.pending) > 5:  # five-deep exp/av stagger
                    s.drain_one()
                tiles_done += 1
                if (fillers and tiles_done % inject_every == 0
                        and tiles_done <= n_tiles - 2):
                    fillers.pop(0)()
            while len(s.pending) > 2:
                s.drain_one()
            prev_stream = s
            if t == 3 or slot == 1:
                for th in fillers:
                    th()
                fillers = []
                qk_flush()
    prev_stream.drain_all()

    # ---- second half of the output projection (slot1 positions) ----
    # pairs share one staging tile and one output DMA: fewer serialized
    # HWDGE windows on the final critical path
    for qg in (4,):
        fin2 = fin_pool.tile([128, 2, 512], F32, tag="f2", name="fin2")
        for sub in range(2):
            psp = ps_s.tile([128, 512], F32, tag="s", name="ps_proj")
            for tt in range(4):
                nc.tensor.matmul(psp[:], outT[tt][qg + sub][:],
                                 projw4[:, tt, :], start=(tt == 0), stop=False)
            nc.tensor.matmul(psp[:], ones1_sb[:, 0:128], biasrow_sb[:],
                             start=False, stop=True)
            nc.scalar.activation(fin2[:, sub], psp[:],
                                 mybir.ActivationFunctionType.Copy)
        nc.sync.dma_start(
            out_d[qg * 128:(qg + 2) * 128, :].rearrange(
                "(s p) c -> p s c", p=128), fin2[:])
    # last two outputs unpaired: their single-width DMAs overlap the copies
    # instead of waiting for both, shortening the final critical chain
    emit_proj(6)
    emit_proj(7)

    ctx.close()


@functools.lru_cache(maxsize=1)
def _graph():
    return _build_graph()


def kernel(x, qkv_w, proj_w, proj_b):
    global LAST_RESULT
    x = np.asarray(x, np.float32)
    qkv_w = np.asarray(qkv_w, np.float32)
    proj_w = np.asarray(proj_w, np.float32)
    proj_b = np.asarray(proj_b, np.float32)

    nc = _graph()
    shared, sin, cos = _shared_inputs(qkv_w, proj_w, proj_b)
    in_maps = [_core_inputs(c, x, shared, sin, cos) for c in range(NCORES)]
    trace = bool(int(os.environ.get("KERNEL_TRACE", "0")))
    res = bass_utils.run_bass_kernel_spmd(nc, in_maps,
                                          core_ids=list(range(NCORES)),
                                          trace=trace)
    LAST_RESULT = res
    out = np.zeros((B, N, C), np.float32)
    for c in range(NCORES):
        b, s = c // 2, c % 2
        blocks = _owned_blocks(s)
        o = np.asarray(res.results[c]["out"], np.float32)
        out[b, blocks[0] * 512:(blocks[0] + 1) * 512] = o[0:512]
        out[b, blocks[1] * 512:(blocks[1] + 1) * 512] = o[512:1024]
    return out


# revision 90
# speedup vs baseline: 1.0121x; 1.0072x over previous
"""Fused multi-head attention layer (RoPE + ALiBi + softmax + out-proj) on 8 TRN2 cores.

Sharding: core c -> (batch b = c//2, query-half s = c%2). Each core owns 1024
queries of its batch (two 512-blocks, interleaved for ALiBi load balance),
computes K/V for all 2048 positions, and writes a disjoint slice of the output.
No collectives. All 8 cores run one SPMD graph; per-core differences (which
query blocks, ALiBi band offsets) are encoded purely in host-prepared data.

v2 optimizations (vs baseline, TimelineSim 222.6us -> 154.3us):
- Score/bias/exp/AV column-trimmed to the ALiBi-needed query prefix per
  (head-pair, key-tile, slot); bias matmuls further restricted to the
  biased suffix via split accumulation groups; av and bias matmuls
  additionally trimmed to each head's own radius (the columns beyond it
  hold exp(<-30)~0 / are never consumed by the trimmed av).
- Both heads of a pair share one [128,1024] score PSUM -> one exp
  activation per (pair, jt, slot), halving ACT fixed overhead.
- AV uses stationary=exp-tile / moving=v (65 cols) instead of streaming
  512 query columns: ~2x less PE stream time; output lands [query, dim].
- Softmax normalization becomes a per-partition tensor_scalar multiply.
- Attention tiles transposed on the DMA engines (xbar), projection runs
  with 128-deep contraction (head pairs packed), proj bias folded in as a
  rank-1 ones-row matmul.
- RoPE rotate_half applied as a +-1 permutation matmul on the projected
  q/k instead of a second rotated-weight projection (drops 96 projection
  matmuls); the psum->sbuf hop also makes the cos-multiply all-bf16.
- Deep software pipelining: score streams run one (t, slot) at a time
  with a five-deep exp/av stagger, chained across stream boundaries; QK
  chunks of the next head pair and the first half of the projection are
  injected as fillers into score-phase PE slack (on the ps_a pool, so
  they never contend with score psums).
- Inputs land in a few large multi-dim DMAs ordered by first consumer;
  the tail transposes bypass the DMA queue via PE transpose; V-phase
  psums alternate between both psum pools so the ring is never
  copy-latency bound; the final output DMAs are paired to halve the
  serialized HWDGE windows on the tail; a throwaway warmup matmul chain
  starts the PE p-state ramp clock early so real matmuls never pay the
  cold-clock penalty; prephase psum->sbuf copies split across DVE/ACT
  and the prephase rotate matmuls deferred one chunk.

(fp8 DoubleRow for the bias matmul halves its PE cost in the cost model
and passes CoreSim numerically, but the axon PJRT execution path fails
on it at runtime, so it stays bf16.)
"""

import functools
import os
import sys

import numpy as np

sys.path.insert(0, "/opt/trn_rl_repo")

import ml_dtypes  # noqa: E402

import concourse.bass as bass  # noqa: E402
import concourse.tile as tile  # noqa: E402
from concourse import bacc, mybir, bass_utils  # noqa: E402

BF16 = mybir.dt.bfloat16
F32 = mybir.dt.float32
NPBF = ml_dtypes.bfloat16

B, N, C, H, D = 4, 2048, 512, 8, 64
NCORES = 8
NQ = 1024            # local queries per core
JT = N // 128        # 16 j-tiles of 128 key positions
T_CUT = 30.0         # ALiBi cutoff in logits: exp(-30) is negligible
SCALE = D ** -0.5

# c8_h = alibi_slope_h * MAX_BIAS = 2^-(h+1) * 8 = 2^(2-h)
C8 = [2.0 ** (2 - h) for h in range(H)]
# band reach (in key positions) per head
RADIUS = [T_CUT / c for c in C8]

# SPMD union bounds over the two cores sharing a slot index:
# slot0 owns blocks {0,1}*512, slot1 owns {2,3}*512.
I0MIN = [0, 1024]
I0MAX = [512, 1536]


def _qhi(t, slot, jt):
    """Needed query-column prefix of the [128 keys x 512 q] tile (pair union)."""
    return max(
        max(0, min(512, 128 * jt + 127 + int(RADIUS[h]) + 1 - I0MIN[slot]))
        for h in (2 * t, 2 * t + 1))


def _qlo(slot, jt):
    """First query column where ALiBi bias can be nonzero (union over cores)."""
    return max(0, min(512, 128 * jt + 1 - I0MAX[slot]))


QHI = [[[_qhi(t, s, jt) for jt in range(JT)] for s in range(2)] for t in range(4)]
QLO = [[_qlo(s, jt) for jt in range(JT)] for s in range(2)]
# per (t, slot, qg): last (smallest) jt in descending order that writes qg
JSTOP = [[[min(jt for jt in range(JT) if QHI[t][s][jt] > 128 * qg)
           for qg in range(4)] for s in range(2)] for t in range(4)]

# per-head widths: av matmuls beyond a head's own radius act on exp(<-30)=~0
# columns and are dropped (contribution ~1e-13; scores/bias/exp keep the
# pair width so the psum/activation structure is untouched)
QHIH = [[[[max(0, min(512, 128 * jt + 127 + int(RADIUS[2 * t + p]) + 1
                      - I0MIN[s])) for p in range(2)] for jt in range(JT)]
         for s in range(2)] for t in range(4)]
JSTOPH = [[[[min(jt for jt in range(JT) if QHIH[t][s][jt][p] > 128 * qg)
             for qg in range(4)] for p in range(2)] for s in range(2)]
          for t in range(4)]

LAST_RESULT = None  # test harness reads exec_time_ns from here


def _owned_blocks(s):
    # 512-query blocks of the batch owned by query-half s (balanced for ALiBi)
    return (0, 3) if s == 0 else (1, 2)


def _rope_tables():
    inv = 1.0 / (10000.0 ** (np.arange(0, D, 2, dtype=np.float32) / D))
    f = np.arange(N, dtype=np.float32)[:, None] * inv[None, :]
    sin = np.concatenate([np.sin(f), np.sin(f)], axis=-1).astype(np.float32)
    cos = np.concatenate([np.cos(f), np.cos(f)], axis=-1).astype(np.float32)
    return sin, cos  # [N, D]


def _shared_inputs(qkv_w, proj_w, proj_b):
    wqT = np.ascontiguousarray(qkv_w[0:C].T) * SCALE       # [C, C]
    wkT = np.ascontiguousarray(qkv_w[C:2 * C].T)
    wvT = np.ascontiguousarray(qkv_w[2 * C:3 * C].T)
    wcat = np.concatenate([wqT, wkT, wvT], axis=1).astype(NPBF)

    ident128 = np.eye(128, dtype=np.float32)

    # rotate_half as a +-1 permutation: out[i] = -in[32+i], out[32+i] = in[i]
    # per 64-dim head; lhsT layout [d_in, d_out].
    rotperm = np.zeros((128, 128), np.float32)
    for hh in range(2):
        for i in range(32):
            rotperm[hh * 64 + 32 + i, hh * 64 + i] = -1.0
            rotperm[hh * 64 + i, hh * 64 + 32 + i] = 1.0

    c8eye = np.zeros((H, 128, 128), np.float32)
    for h in range(H):
        np.fill_diagonal(c8eye[h], C8[h])

    sin, cos = _rope_tables()
    cos2k = np.tile(cos.T, (2, 1))                         # [128, N]
    sin2k = np.tile(sin.T, (2, 1))
    return {
        "wcat": wcat,
        "rotperm": rotperm.astype(NPBF),
        "ident128": ident128.astype(NPBF),
        "c8eye": c8eye.astype(NPBF),
        "projwt": np.ascontiguousarray(proj_w.T).astype(NPBF),
        "biasrow": proj_b[None, :].astype(NPBF),
        "cos2k": cos2k.astype(NPBF), "sin2k": sin2k.astype(NPBF),
    }, sin, cos


def _pats_for(i0):
    jl = np.arange(128, dtype=np.float32)[:, None]
    il = np.arange(512, dtype=np.float32)[None, :]
    return [np.minimum((jt * 128 + jl) - (i0 + il), 0.0).astype(NPBF)
            for jt in range(16)]


def _core_inputs(c, x, shared, sin, cos):
    b, s = c // 2, c % 2
    blocks = _owned_blocks(s)
    gi = np.concatenate([np.arange(blk * 512, (blk + 1) * 512) for blk in blocks])

    xt = np.ascontiguousarray(x[b].T)                      # [C, N]
    xtq = np.ascontiguousarray(x[b][gi].T)                 # [C, NQ]

    cos2q = np.tile(cos[gi].T, (2, 1))                     # [128, NQ]
    sin2q = np.tile(sin[gi].T, (2, 1))

    pats0 = np.stack(_pats_for(blocks[0] * 512)[:8])
    pats1 = np.stack(_pats_for(blocks[1] * 512))

    return {
        "xt": xt.astype(NPBF),
        "xtq": xtq.astype(NPBF),
        "cos2q": cos2q.astype(NPBF), "sin2q": sin2q.astype(NPBF),
        "pats0": pats0,
        "pats1": pats1,
        **shared,
    }


def _build_graph():
    nc = bacc.Bacc("TRN2", target_bir_lowering=False, debug=False,
                   num_devices=NCORES)

    xt_d = nc.dram_tensor("xt", [C, N], BF16, kind="ExternalInput").ap()
    xtq_d = nc.dram_tensor("xtq", [C, NQ], BF16, kind="ExternalInput").ap()
    wcat_d = nc.dram_tensor("wcat", [C, 3 * C], BF16, kind="ExternalInput").ap()
    rotperm_d = nc.dram_tensor("rotperm", [128, 128], BF16, kind="ExternalInput").ap()
    ident_d = nc.dram_tensor("ident128", [128, 128], BF16, kind="ExternalInput").ap()
    cos2q_d = nc.dram_tensor("cos2q", [128, NQ], BF16, kind="ExternalInput").ap()
    sin2q_d = nc.dram_tensor("sin2q", [128, NQ], BF16, kind="ExternalInput").ap()
    cos2k_d = nc.dram_tensor("cos2k", [128, N], BF16, kind="ExternalInput").ap()
    sin2k_d = nc.dram_tensor("sin2k", [128, N], BF16, kind="ExternalInput").ap()
    pats0_d = nc.dram_tensor("pats0", [8, 128, 512], BF16, kind="ExternalInput").ap()
    pats1_d = nc.dram_tensor("pats1", [16, 128, 512], BF16, kind="ExternalInput").ap()
    c8eye_d = nc.dram_tensor("c8eye", [H, 128, 128], BF16, kind="ExternalInput").ap()
    projwt_d = nc.dram_tensor("projwt", [C, C], BF16, kind="ExternalInput").ap()
    biasrow_d = nc.dram_tensor("biasrow", [1, 512], BF16, kind="ExternalInput").ap()
    out_d = nc.dram_tensor("out", [NQ, C], F32, kind="ExternalOutput").ap()

    with tile.TileContext(nc) as tc:
        _body(nc, tc, xt_d, xtq_d, wcat_d, rotperm_d, ident_d, cos2q_d,
              sin2q_d, cos2k_d, sin2k_d, pats0_d, pats1_d, c8eye_d, projwt_d,
              biasrow_d, out_d)
    nc.compile()
    return nc


def _body(nc, tc, xt_d, xtq_d, wcat_d, rotperm_d, ident_d, cos2q_d,
          sin2q_d, cos2k_d, sin2k_d, pats0_d, pats1_d, c8eye_d, projwt_d,
          biasrow_d, out_d):
    from contextlib import ExitStack
    ctx = ExitStack()
    persist = ctx.enter_context(tc.tile_pool(name="persist", bufs=1))
    tmp_pool = ctx.enter_context(tc.tile_pool(name="ropetmp", bufs=6))
    exp_pool = ctx.enter_context(tc.tile_pool(name="exp", bufs=10))
    fin_pool = ctx.enter_context(tc.tile_pool(name="final", bufs=2))
    att_pool = ctx.enter_context(tc.tile_pool(name="att", bufs=8))
    rec_pool = ctx.enter_context(tc.tile_pool(name="rec", bufs=4))
    # PSUM: ps_s = 2 bufs x [128,1024] f32 (2 banks each); ps_a = 4 bufs x
    # [128,512] f32 (1 bank each) shared by QKV-phase psums and AV accums.
    ps_s = ctx.enter_context(tc.tile_pool(name="ps_s", bufs=2, space="PSUM"))
    ps_a = ctx.enter_context(tc.tile_pool(name="ps_a", bufs=4, space="PSUM"))

    def ptile(shape, dtype, tag):
        return persist.tile(shape, dtype, tag=tag, name=tag)

    Exp = mybir.ActivationFunctionType.Exp

    # PE p-state warmup: the cost model ramps 0.65 -> 1.2 -> 2.4 GHz over
    # ~3us of continuous execution. A chain of throwaway matmuls (dependent
    # only on an early memset) starts the ramp clock at ~0.3us so the first
    # real V matmuls already run at full clock.
    ones1_sb = persist.tile([1, 512], BF16, tag="ones1", name="ones1")
    nc.vector.memset(ones1_sb[:], 1.0)
    warm = ps_s.tile([128, 512], F32, tag="s", name="ps_warm")
    for _ in range(6):
        nc.tensor.matmul(warm[:], ones1_sb[:, 0:128], ones1_sb[:],
                         start=True, stop=True)

    # ---- persistent SBUF tiles + input DMAs, emitted in consumer order ----
    # channel blocks live in a middle free dim so each tensor loads in one
    # (or a few) large DMAs instead of 4x4 small ones
    w2 = ptile([128, 4, 3 * C], BF16, "w2")
    xt2 = ptile([128, 4, N], BF16, "xt2")
    xtq2 = ptile([128, 4, NQ], BF16, "xtq2")
    wcat_r = wcat_d.rearrange("(i p) c -> p i c", p=128)
    xt_r = xt_d.rearrange("(i p) n -> p i n", p=128)

    # DMA order follows consumption order: V (position-descending) needs
    # w-v + xt blk3 first; K projections need wk + k tables; then Q inputs;
    # then bias patterns (jt-descending, slot1 first); proj weights last.
    for i in range(4):  # first consumers: small DMAs for fast first arrival
        nc.sync.dma_start(w2[:, i, 2 * C:3 * C], wcat_r[:, i, 2 * C:3 * C])
        nc.sync.dma_start(xt2[:, i, 1536:2048], xt_r[:, i, 1536:2048])
    for blk in (2, 1, 0):  # V consumes position blocks descending
        nc.sync.dma_start(xt2[:, :, blk * 512:(blk + 1) * 512],
                          xt_r[:, :, blk * 512:(blk + 1) * 512])
    nc.sync.dma_start(w2[:, :, C:2 * C], wcat_r[:, :, C:2 * C])
    cos2k = ptile([128, N], BF16, "cos2k")
    nc.sync.dma_start(cos2k[:], cos2k_d[:])
    sin2k = ptile([128, N], BF16, "sin2k")
    nc.sync.dma_start(sin2k[:], sin2k_d[:])
    rotperm_sb = ptile([128, 128], BF16, "rotperm")
    nc.sync.dma_start(rotperm_sb[:], rotperm_d[:])
    cos2q = ptile([128, NQ], BF16, "cos2q")
    nc.sync.dma_start(cos2q[:], cos2q_d[:])
    sin2q = ptile([128, NQ], BF16, "sin2q")
    nc.sync.dma_start(sin2q[:], sin2q_d[:])
    nc.sync.dma_start(w2[:, :, 0:C], wcat_r[:, :, 0:C])
    nc.sync.dma_start(xtq2[:, :, :],
                      xtq_d.rearrange("(i p) n -> p i n", p=128))
    ident_sb = ptile([128, 128], BF16, "ident128")
    nc.sync.dma_start(ident_sb[:], ident_d[:])

    c8eye2 = ptile([128, H, 128], BF16, "c8eye2")
    nc.sync.dma_start(c8eye2[:], c8eye_d.rearrange("h p e -> p h e"))
    pats1_t = ptile([128, 16, 512], BF16, "pats1t")
    pats1_r = pats1_d.rearrange("j p q -> p j q")
    for j0 in (12, 8, 4, 0):
        nc.sync.dma_start(pats1_t[:, j0:j0 + 4, :], pats1_r[:, j0:j0 + 4, :])
    pats0_t = ptile([128, 8, 512], BF16, "pats0t")
    pats0_r = pats0_d.rearrange("j p q -> p j q")
    for j0 in (4, 0):
        nc.sync.dma_start(pats0_t[:, j0:j0 + 4, :], pats0_r[:, j0:j0 + 4, :])
    projw4 = ptile([128, 4, 512], BF16, "projw4")
    nc.sync.dma_start(projw4[:], projwt_d.rearrange("(t p) c -> p t c", p=128))
    biasrow_sb = ptile([1, 512], BF16, "biasrow")
    nc.sync.dma_start(biasrow_sb[:], biasrow_d[:])

    q2_sb = [ptile([128, NQ], BF16, f"q2_{t}") for t in range(4)]
    k2_sb = [ptile([128, N], BF16, f"k2_{t}") for t in range(4)]
    v_sb = [ptile([128, H * 65], BF16, f"v_{nt}") for nt in range(JT)]
    # transposed attention tiles [dims(2 heads x 64), 128 queries] per (t, qg)
    outT = [[ptile([128, 128], BF16, f"oT{t}_{qg}") for qg in range(8)]
            for t in range(4)]

    # ---- V projection, position-descending (av streams consume jt=15 first)
    def emit_v(nt):
        # alternate V psums between the (idle) score pool and ps_a: more
        # slots in flight -> the psum ring is no longer copy-latency bound
        pool = ps_s if nt % 2 == 0 else ps_a
        tag = "s" if nt % 2 == 0 else "a"
        psv = pool.tile([128, 512], F32, tag=tag, name="ps_v")
        for ci in range(4):
            nc.tensor.matmul(
                psv[:], xt2[:, ci, nt * 128:(nt + 1) * 128],
                w2[:, ci, 2 * C:3 * C],
                start=(ci == 0), stop=(ci == 3))
        vdst = v_sb[nt].rearrange("p (h e) -> p h e", e=65)
        # DVE is the prephase pacer and ACT is idle there: alternate the
        # psum->sbuf copies between them
        if nt % 2 == 0:
            nc.vector.tensor_copy(vdst[:, :, 0:64],
                                   psv.rearrange("p (h e) -> p h e", e=64))
        else:
            nc.scalar.activation(vdst[:, :, 0:64],
                                 psv.rearrange("p (h e) -> p h e", e=64),
                                 mybir.ActivationFunctionType.Copy)
        nc.gpsimd.memset(vdst[:, :, 64:65], 1.0)

    # ---- Q/K projections + RoPE (psums from ps_a: no contention with the
    # score psums when chunks are injected into the score phase) ----
    # QK chunk, software-pipelined in two parts so the rotate matmul never
    # waits in-order on its own chunk's psum->sbuf copy.
    qk_pend = []

    def qk_part2(st):
        qsb, cos_sb, sin_sb, dst_sb, c0 = st
        ps_r = ps_a.tile([128, 512], F32, tag="a", name="ps_r")
        nc.tensor.matmul(ps_r[:], rotperm_sb[:], qsb[:], start=True, stop=True)
        tc_c = tmp_pool.tile([128, 512], BF16, tag="rt", name="rt_c")
        nc.vector.tensor_mul(tc_c[:], qsb[:], cos_sb[:, c0:c0 + 512])
        tc_u = tmp_pool.tile([128, 512], F32, tag="ru", name="rt_u")
        nc.vector.tensor_mul(tc_u[:], ps_r[:], sin_sb[:, c0:c0 + 512])
        nc.gpsimd.tensor_add(dst_sb[:, c0:c0 + 512], tc_c[:], tc_u[:])

    def qk_flush():
        while qk_pend:
            qk_part2(qk_pend.pop(0))

    def qk_chunk(dt_tile, w_off, rhs_sb, cos_sb, sin_sb, dst_sb, c0,
                 act_copy=False, defer=False):
        # one projection matmul set; rotate_half applied afterwards as a
        # +-1 permutation matmul (replaces the rotated-weight projection)
        ps_q = ps_a.tile([128, 512], F32, tag="a", name="ps_q")
        for ci in range(4):
            nc.tensor.matmul(
                ps_q[:],
                w2[:, ci, w_off + dt_tile * 128: w_off + (dt_tile + 1) * 128],
                rhs_sb[:, ci, c0:c0 + 512],
                start=(ci == 0), stop=(ci == 3))
        qsb = tmp_pool.tile([128, 512], BF16, tag="qs", name="qsb")
        if act_copy:  # ACT is idle in the upfront QK phase; DVE is the pacer
            nc.scalar.activation(qsb[:], ps_q[:],
                                 mybir.ActivationFunctionType.Copy)
        else:
            nc.vector.tensor_copy(qsb[:], ps_q[:])
        if defer:  # prephase: rot matmul runs under the next chunk's cover
            qk_pend.append((qsb, cos_sb, sin_sb, dst_sb, c0))
            if len(qk_pend) > 1:
                qk_part2(qk_pend.pop(0))
        else:
            qk_part2((qsb, cos_sb, sin_sb, dst_sb, c0))

    def qk_fillers(t, act_copy=False, defer=False):
        """QK work for head pair t as thunks: k chunks (desc), then q."""
        thunks = []
        for ch in range(N // 512 - 1, -1, -1):
            thunks.append(lambda ch=ch: qk_chunk(
                t, C, xt2, cos2k, sin2k, k2_sb[t], ch * 512, act_copy, defer))
        for ch in range(NQ // 512):
            thunks.append(lambda ch=ch: qk_chunk(
                t, 0, xtq2, cos2q, sin2q, q2_sb[t], ch * 512, act_copy, defer))
        return thunks

    # Interleave QK(t0) chunks with the V tail so the PE rides out the
    # later xt-block DMAs instead of stalling on them.
    qk0 = qk_fillers(0, act_copy=True, defer=True)  # k desc, then q
    for nt in range(15, 7, -1):
        emit_v(nt)
    qk0.pop(0)()
    qk0.pop(0)()
    for nt in range(7, 3, -1):
        emit_v(nt)
    qk0.pop(0)()
    for nt in range(3, -1, -1):
        emit_v(nt)
    for th in qk0:
        th()
    qk_flush()

    # ---- scores + ALiBi bias + exp + attn@v (stationary-exp orientation) ----
    # The two 512-query slots of a head pair run as interleaved streams so
    # the PE always has the other stream's score matmuls to chew on while
    # ACT computes this stream's exp (in-order PE would otherwise stall on
    # every av group).
    class Stream:
        def __init__(self, t, slot):
            self.t, self.slot = t, slot
            self.jts = [jt for jt in range(JT - 1, -1, -1)
                        if QHI[t][slot][jt] > 0]
            self.pats = pats1_t if slot == 1 else pats0_t
            self.avp = [ps_a.tile([128, 4 * 65], F32, tag="a",
                                  name=f"av{t}_{slot}_{p}") for p in range(2)]
            self.ks = 0          # score tiles emitted
            self.ka = 0          # av groups emitted
            self.pending = []    # (jt, qhi, et) awaiting av emission
            self.normed = False

        def scores_left(self):
            return self.ks < len(self.jts)

        def drain_one(self):
            if self.pending:
                self.emit_av()
            if self.ka == len(self.jts) and not self.normed:
                self.normed = True
                self.emit_normalize()

        def drain_all(self):
            while self.pending:
                self.drain_one()
            self.drain_one()

        def emit_scores(self):
            t, slot = self.t, self.slot
            jt = self.jts[self.ks]
            qhi = QHI[t][slot][jt]
            qlo = min(QLO[slot][jt], qhi)
            ps = ps_s.tile([128, 1024], F32, tag="s", name="ps_sc")
            for p in range(2):
                h = 2 * t + p
                ks = k2_sb[t][64 * p:64 * (p + 1), jt * 128:(jt + 1) * 128]
                qbase = slot * 512
                # bias only matters up to this head's own radius: av skips
                # the columns beyond it, so their raw (unbiased) scores are
                # computed (the paired exp reads them) but never consumed
                bhi = min(qhi, QHIH[t][slot][jt][p])
                # One psum "zero region" (bank) per head-half: the first
                # matmul starts (lazy-zeroes) it, the last one stops it.
                if qlo >= bhi:  # no biased columns this head consumes
                    nc.tensor.matmul(
                        ps[:, p * 512:p * 512 + qhi], ks,
                        q2_sb[t][64 * p:64 * (p + 1), qbase:qbase + qhi],
                        start=True, stop=True, tile_position=(64 * p, 0))
                else:
                    if qlo > 0:
                        nc.tensor.matmul(
                            ps[:, p * 512:p * 512 + qlo], ks,
                            q2_sb[t][64 * p:64 * (p + 1), qbase:qbase + qlo],
                            start=True, stop=False, tile_position=(64 * p, 0))
                    nc.tensor.matmul(
                        ps[:, p * 512 + qlo:p * 512 + qhi], ks,
                        q2_sb[t][64 * p:64 * (p + 1), qbase + qlo:qbase + qhi],
                        start=(qlo == 0), stop=False, tile_position=(64 * p, 0))
                    nc.tensor.matmul(
                        ps[:, p * 512 + qlo:p * 512 + bhi], c8eye2[:, h, :],
                        self.pats[:, jt, qlo:bhi],
                        start=False, stop=True, tile_position=(0, 0))
            et = exp_pool.tile([128, 1024], BF16, tag="e", name="et")
            if qhi == 512:
                nc.scalar.activation(et[:], ps[:], Exp)
            else:
                psv_ap = ps.rearrange("p (h q) -> p h q", h=2)[:, :, 0:qhi]
                etv_ap = et.rearrange("p (h q) -> p h q", h=2)[:, :, 0:qhi]
                nc.scalar.activation(etv_ap, psv_ap, Exp)
            self.pending.append((jt, qhi, et))
            self.ks += 1

        def emit_av(self):
            t, slot = self.t, self.slot
            jt, qhi, et = self.pending.pop(0)
            first = (self.ka == 0)
            for p in range(2):
                h = 2 * t + p
                qhi_p = min(qhi, QHIH[t][slot][jt][p])
                vs = v_sb[jt][:, h * 65:(h + 1) * 65]
                for qg in range(4):
                    qw = min(128, qhi_p - 128 * qg)
                    if qw <= 0:
                        break
                    nc.tensor.matmul(
                        self.avp[p][0:qw, qg * 65:(qg + 1) * 65],
                        et[:, p * 512 + qg * 128:p * 512 + qg * 128 + qw],
                        vs,
                        start=(first and qg == 0),
                        stop=(jt == JSTOPH[t][slot][p][qg]),
                        skip_group_check=True)
            self.ka += 1

        def emit_normalize(self):
            t, slot = self.t, self.slot
            recs = []
            for p in range(2):
                rec = rec_pool.tile([128, 4], F32, tag="r", name="rec")
                nc.vector.reciprocal(
                    rec[:],
                    self.avp[p].rearrange("p (g e) -> p g e", e=65)[:, :, 64])
                recs.append(rec)
            for qg in range(4):
                att = att_pool.tile([128, 128], BF16, tag="t", name="att")
                for p in range(2):
                    nc.vector.tensor_scalar_mul(
                        att[:, p * 64:(p + 1) * 64],
                        self.avp[p][:, qg * 65:qg * 65 + 64],
                        recs[p][:, qg:qg + 1])
                if t == 3 and slot == 1:
                    # tail: PE transpose (+DVE copy) beats the serialized
                    # HWDGE xbar path on the critical path to the projection
                    ps_t = ps_a.tile([128, 128], BF16, tag="a", name="ps_t")
                    nc.tensor.transpose(ps_t[:], att[:], ident_sb[:])
                    nc.vector.tensor_copy(outT[t][slot * 4 + qg][:], ps_t[:])
                else:
                    nc.sync.dma_start_transpose(outT[t][slot * 4 + qg][:],
                                                att[:])

    def emit_proj(qg):
        psp = ps_a.tile([128, 512], F32, tag="a", name="ps_proj")
        for tt in range(4):
            nc.tensor.matmul(psp[:], outT[tt][qg][:], projw4[:, tt, :],
                             start=(tt == 0), stop=False)
        nc.tensor.matmul(psp[:], ones1_sb[:, 0:128], biasrow_sb[:],
                         start=False, stop=True)
        fin = fin_pool.tile([128, 512], F32, tag="f", name="fin")
        nc.scalar.activation(fin[:], psp[:], mybir.ActivationFunctionType.Copy)
        nc.sync.dma_start(out_d[qg * 128:(qg + 1) * 128, :], fin[:])

    prev_stream = None
    for t in range(4):
        # Fillers injected into score-phase PE slack (they use ps_a slots,
        # not the score psums): QK chunks of the next head pair, and for the
        # last pair's slot1 the first half of the output projection (whose
        # outT inputs - slot0 of every pair - are complete by then).
        for slot in range(2):
            if t < 3:
                fillers = qk_fillers(t + 1) if slot == 0 else fillers
            else:
                fillers = [] if slot == 0 else [
                    (lambda qg=qg: emit_proj(qg)) for qg in range(4)]
            n_tiles = len([jt for jt in range(JT) if QHI[t][slot][jt] > 0])
            inject_every = max(3, n_tiles // max(1, len(fillers) or 1))
            tiles_done = 0
            s = Stream(t, slot)
            # prologue: two tiles in flight, then drain the previous
            # stream's tail under their cover (hides its last exp latency)
            s.emit_scores()
            if s.scores_left():
                s.emit_scores()
            if prev_stream is not None:
                prev_stream.drain_all()
            tiles_done = s.ks
            while s.scores_left():
                s.emit_scores()
                while len(s.pending) > 5:  # five-deep exp/av stagger
                    s.drain_one()
                tiles_done += 1
                if (fillers and tiles_done % inject_every == 0
                        and tiles_done <= n_tiles - 2):
                    fillers.pop(0)()
            while len(s.pending) > 2:
                s.drain_one()
            prev_stream = s
            if t == 3 or slot == 1:
                for th in fillers:
                    th()
                fillers = []
                qk_flush()
    prev_stream.drain_all()

    # ---- second half of the output projection (slot1 positions) ----
    # pairs share one staging tile and one output DMA: fewer serialized
    # HWDGE windows on the final critical path
    for qg in (4,):
        fin2 = fin_pool.tile([128, 2, 512], F32, tag="f2", name="fin2")
        for sub in range(2):
            psp = ps_s.tile([128, 512], F32, tag="s", name="ps_proj")
            for tt in range(4):
                nc.tensor.matmul(psp[:], outT[tt][qg + sub][:],
                                 projw4[:, tt, :], start=(tt == 0), stop=False)
            nc.tensor.matmul(psp[:], ones1_sb[:, 0:128], biasrow_sb[:],
                             start=False, stop=True)
            nc.scalar.activation(fin2[:, sub], psp[:],
                                 mybir.ActivationFunctionType.Copy)
        nc.sync.dma_start(
            out_d[qg * 128:(qg + 2) * 128, :].rearrange(
                "(s p) c -> p s c", p=128), fin2[:])
    # last two outputs unpaired: their single-width DMAs overlap the copies
    # instead of waiting for both, shortening the final critical chain
    emit_proj(6)
    emit_proj(7)

    ctx.close()


@functools.lru_cache(maxsize=1)
def _graph():
    return _build_graph()


def kernel(x, qkv_w, proj_w, proj_b):
    global LAST_RESULT
    x = np.asarray(x, np.float32)
    qkv_w = np.asarray(qkv_w, np.float32)
    proj_w = np.asarray(proj_w, np.float32)
    proj_b = np.asarray(proj_b, np.float32)

    nc = _graph()
    shared, sin, cos = _shared_inputs(qkv_w, proj_w, proj_b)
    in_maps = [_core_inputs(c, x, shared, sin, cos) for c in range(NCORES)]
    trace = bool(int(os.environ.get("KERNEL_TRACE", "0")))
    res = bass_utils.run_bass_kernel_spmd(nc, in_maps,
                                          core_ids=list(range(NCORES)),
                                          trace=trace)
    LAST_RESULT = res
    out = np.zeros((B, N, C), np.float32)
    for c in range(NCORES):
        b, s = c // 2, c % 2
        blocks = _owned_blocks(s)
        o = np.asarray(res.results[c]["out"], np.float32)
        out[b, blocks[0] * 512:(blocks[0] + 1) * 512] = o[0:512]
        out[b, blocks[1] * 512:(blocks[1] + 1) * 512] = o[512:1024]
    return out


# revision 94
# speedup vs baseline: 1.0160x; 1.0039x over previous
"""Fused multi-head attention layer (RoPE + ALiBi + softmax + out-proj) on 8 TRN2 cores.

Sharding: core c -> (batch b = c//2, query-half s = c%2). Each core owns 1024
queries of its batch (two 512-blocks, interleaved for ALiBi load balance),
computes K/V for all 2048 positions, and writes a disjoint slice of the output.
No collectives. All 8 cores run one SPMD graph; per-core differences (which
query blocks, ALiBi band offsets) are encoded purely in host-prepared data.

v2 optimizations (vs baseline, TimelineSim 222.6us -> 154.3us):
- Score/bias/exp/AV column-trimmed to the ALiBi-needed query prefix per
  (head-pair, key-tile, slot); bias matmuls further restricted to the
  biased suffix via split accumulation groups; av and bias matmuls
  additionally trimmed to each head's own radius (the columns beyond it
  hold exp(<-30)~0 / are never consumed by the trimmed av).
- Both heads of a pair share one [128,1024] score PSUM -> one exp
  activation per (pair, jt, slot), halving ACT fixed overhead.
- AV uses stationary=exp-tile / moving=v (65 cols) instead of streaming
  512 query columns: ~2x less PE stream time; output lands [query, dim].
- Softmax normalization becomes a per-partition tensor_scalar multiply.
- Attention tiles transposed on the DMA engines (xbar), projection runs
  with 128-deep contraction (head pairs packed), proj bias folded in as a
  rank-1 ones-row matmul.
- RoPE rotate_half applied as a +-1 permutation matmul on the projected
  q/k instead of a second rotated-weight projection (drops 96 projection
  matmuls); the psum->sbuf hop also makes the cos-multiply all-bf16.
- Deep software pipelining: score streams run one (t, slot) at a time
  with a five-deep exp/av stagger, chained across stream boundaries; QK
  chunks of the next head pair and the first half of the projection are
  injected as fillers into score-phase PE slack (on the ps_a pool, so
  they never contend with score psums).
- Inputs land in a few large multi-dim DMAs ordered by first consumer;
  the tail transposes bypass the DMA queue via PE transpose; V-phase
  psums alternate between both psum pools so the ring is never
  copy-latency bound; the final output DMAs are paired to halve the
  serialized HWDGE windows on the tail; a throwaway warmup matmul chain
  starts the PE p-state ramp clock early so real matmuls never pay the
  cold-clock penalty; prephase psum->sbuf copies split across DVE/ACT
  and the prephase rotate matmuls deferred one chunk.

(fp8 DoubleRow for the bias matmul halves its PE cost in the cost model
and passes CoreSim numerically, but the axon PJRT execution path fails
on it at runtime, so it stays bf16.)
"""

import functools
import os
import sys

import numpy as np

sys.path.insert(0, "/opt/trn_rl_repo")

import ml_dtypes  # noqa: E402

import concourse.bass as bass  # noqa: E402
import concourse.tile as tile  # noqa: E402
from concourse import bacc, mybir, bass_utils  # noqa: E402

BF16 = mybir.dt.bfloat16
F32 = mybir.dt.float32
NPBF = ml_dtypes.bfloat16

B, N, C, H, D = 4, 2048, 512, 8, 64
NCORES = 8
NQ = 1024            # local queries per core
JT = N // 128        # 16 j-tiles of 128 key positions
T_CUT = 30.0         # ALiBi cutoff in logits: exp(-30) is negligible
SCALE = D ** -0.5

# c8_h = alibi_slope_h * MAX_BIAS = 2^-(h+1) * 8 = 2^(2-h)
C8 = [2.0 ** (2 - h) for h in range(H)]
# band reach (in key positions) per head
RADIUS = [T_CUT / c for c in C8]

# SPMD union bounds over the two cores sharing a slot index:
# slot0 owns blocks {0,1}*512, slot1 owns {2,3}*512.
I0MIN = [0, 1024]
I0MAX = [512, 1536]


def _qhi(t, slot, jt):
    """Needed query-column prefix of the [128 keys x 512 q] tile (pair union)."""
    return max(
        max(0, min(512, 128 * jt + 127 + int(RADIUS[h]) + 1 - I0MIN[slot]))
        for h in (2 * t, 2 * t + 1))


def _qlo(slot, jt):
    """First query column where ALiBi bias can be nonzero (union over cores)."""
    return max(0, min(512, 128 * jt + 1 - I0MAX[slot]))


QHI = [[[_qhi(t, s, jt) for jt in range(JT)] for s in range(2)] for t in range(4)]
QLO = [[_qlo(s, jt) for jt in range(JT)] for s in range(2)]
# per (t, slot, qg): last (smallest) jt in descending order that writes qg
JSTOP = [[[min(jt for jt in range(JT) if QHI[t][s][jt] > 128 * qg)
           for qg in range(4)] for s in range(2)] for t in range(4)]

# per-head widths: av matmuls beyond a head's own radius act on exp(<-30)=~0
# columns and are dropped (contribution ~1e-13; scores/bias/exp keep the
# pair width so the psum/activation structure is untouched)
QHIH = [[[[max(0, min(512, 128 * jt + 127 + int(RADIUS[2 * t + p]) + 1
                      - I0MIN[s])) for p in range(2)] for jt in range(JT)]
         for s in range(2)] for t in range(4)]
JSTOPH = [[[[min(jt for jt in range(JT) if QHIH[t][s][jt][p] > 128 * qg)
             for qg in range(4)] for p in range(2)] for s in range(2)]
          for t in range(4)]

LAST_RESULT = None  # test harness reads exec_time_ns from here


def _owned_blocks(s):
    # 512-query blocks of the batch owned by query-half s (balanced for ALiBi)
    return (0, 3) if s == 0 else (1, 2)


def _rope_tables():
    inv = 1.0 / (10000.0 ** (np.arange(0, D, 2, dtype=np.float32) / D))
    f = np.arange(N, dtype=np.float32)[:, None] * inv[None, :]
    sin = np.concatenate([np.sin(f), np.sin(f)], axis=-1).astype(np.float32)
    cos = np.concatenate([np.cos(f), np.cos(f)], axis=-1).astype(np.float32)
    return sin, cos  # [N, D]


def _shared_inputs(qkv_w, proj_w, proj_b):
    wqT = np.ascontiguousarray(qkv_w[0:C].T) * SCALE       # [C, C]
    wkT = np.ascontiguousarray(qkv_w[C:2 * C].T)
    wvT = np.ascontiguousarray(qkv_w[2 * C:3 * C].T)
    wcat = np.concatenate([wqT, wkT, wvT], axis=1).astype(NPBF)

    ident128 = np.eye(128, dtype=np.float32)

    # rotate_half as a +-1 permutation: out[i] = -in[32+i], out[32+i] = in[i]
    # per 64-dim head; lhsT layout [d_in, d_out].
    rotperm = np.zeros((128, 128), np.float32)
    for hh in range(2):
        for i in range(32):
            rotperm[hh * 64 + 32 + i, hh * 64 + i] = -1.0
            rotperm[hh * 64 + i, hh * 64 + 32 + i] = 1.0

    c8eye = np.zeros((H, 128, 128), np.float32)
    for h in range(H):
        np.fill_diagonal(c8eye[h], C8[h])

    sin, cos = _rope_tables()
    cos2k = np.tile(cos.T, (2, 1))                         # [128, N]
    sin2k = np.tile(sin.T, (2, 1))
    return {
        "wcat": wcat,
        "rotperm": rotperm.astype(NPBF),
        "ident128": ident128.astype(NPBF),
        "c8eye": c8eye.astype(NPBF),
        "projwt": np.ascontiguousarray(proj_w.T).astype(NPBF),
        "biasrow": proj_b[None, :].astype(NPBF),
        "cos2k": cos2k.astype(NPBF), "sin2k": sin2k.astype(NPBF),
    }, sin, cos


def _pats_for(i0):
    jl = np.arange(128, dtype=np.float32)[:, None]
    il = np.arange(512, dtype=np.float32)[None, :]
    return [np.minimum((jt * 128 + jl) - (i0 + il), 0.0).astype(NPBF)
            for jt in range(16)]


def _core_inputs(c, x, shared, sin, cos):
    b, s = c // 2, c % 2
    blocks = _owned_blocks(s)
    gi = np.concatenate([np.arange(blk * 512, (blk + 1) * 512) for blk in blocks])

    xt = np.ascontiguousarray(x[b].T)                      # [C, N]
    xtq = np.ascontiguousarray(x[b][gi].T)                 # [C, NQ]

    cos2q = np.tile(cos[gi].T, (2, 1))                     # [128, NQ]
    sin2q = np.tile(sin[gi].T, (2, 1))

    pats0 = np.stack(_pats_for(blocks[0] * 512)[:8])
    pats1 = np.stack(_pats_for(blocks[1] * 512))

    return {
        "xt": xt.astype(NPBF),
        "xtq": xtq.astype(NPBF),
        "cos2q": cos2q.astype(NPBF), "sin2q": sin2q.astype(NPBF),
        "pats0": pats0,
        "pats1": pats1,
        **shared,
    }


def _build_graph():
    nc = bacc.Bacc("TRN2", target_bir_lowering=False, debug=False,
                   num_devices=NCORES)

    xt_d = nc.dram_tensor("xt", [C, N], BF16, kind="ExternalInput").ap()
    xtq_d = nc.dram_tensor("xtq", [C, NQ], BF16, kind="ExternalInput").ap()
    wcat_d = nc.dram_tensor("wcat", [C, 3 * C], BF16, kind="ExternalInput").ap()
    rotperm_d = nc.dram_tensor("rotperm", [128, 128], BF16, kind="ExternalInput").ap()
    ident_d = nc.dram_tensor("ident128", [128, 128], BF16, kind="ExternalInput").ap()
    cos2q_d = nc.dram_tensor("cos2q", [128, NQ], BF16, kind="ExternalInput").ap()
    sin2q_d = nc.dram_tensor("sin2q", [128, NQ], BF16, kind="ExternalInput").ap()
    cos2k_d = nc.dram_tensor("cos2k", [128, N], BF16, kind="ExternalInput").ap()
    sin2k_d = nc.dram_tensor("sin2k", [128, N], BF16, kind="ExternalInput").ap()
    pats0_d = nc.dram_tensor("pats0", [8, 128, 512], BF16, kind="ExternalInput").ap()
    pats1_d = nc.dram_tensor("pats1", [16, 128, 512], BF16, kind="ExternalInput").ap()
    c8eye_d = nc.dram_tensor("c8eye", [H, 128, 128], BF16, kind="ExternalInput").ap()
    projwt_d = nc.dram_tensor("projwt", [C, C], BF16, kind="ExternalInput").ap()
    biasrow_d = nc.dram_tensor("biasrow", [1, 512], BF16, kind="ExternalInput").ap()
    out_d = nc.dram_tensor("out", [NQ, C], F32, kind="ExternalOutput").ap()

    with tile.TileContext(nc) as tc:
        _body(nc, tc, xt_d, xtq_d, wcat_d, rotperm_d, ident_d, cos2q_d,
              sin2q_d, cos2k_d, sin2k_d, pats0_d, pats1_d, c8eye_d, projwt_d,
              biasrow_d, out_d)
    nc.compile()
    return nc


def _body(nc, tc, xt_d, xtq_d, wcat_d, rotperm_d, ident_d, cos2q_d,
          sin2q_d, cos2k_d, sin2k_d, pats0_d, pats1_d, c8eye_d, projwt_d,
          biasrow_d, out_d):
    from contextlib import ExitStack
    ctx = ExitStack()
    persist = ctx.enter_context(tc.tile_pool(name="persist", bufs=1))
    tmp_pool = ctx.enter_context(tc.tile_pool(name="ropetmp", bufs=6))
    exp_pool = ctx.enter_context(tc.tile_pool(name="exp", bufs=10))
    fin_pool = ctx.enter_context(tc.tile_pool(name="final", bufs=2))
    att_pool = ctx.enter_context(tc.tile_pool(name="att", bufs=8))
    rec_pool = ctx.enter_context(tc.tile_pool(name="rec", bufs=4))
    # PSUM: ps_s = 2 bufs x [128,1024] f32 (2 banks each); ps_a = 4 bufs x
    # [128,512] f32 (1 bank each) shared by QKV-phase psums and AV accums.
    ps_s = ctx.enter_context(tc.tile_pool(name="ps_s", bufs=2, space="PSUM"))
    ps_a = ctx.enter_context(tc.tile_pool(name="ps_a", bufs=4, space="PSUM"))

    def ptile(shape, dtype, tag):
        return persist.tile(shape, dtype, tag=tag, name=tag)

    Exp = mybir.ActivationFunctionType.Exp

    # PE p-state warmup: the cost model ramps 0.65 -> 1.2 -> 2.4 GHz over
    # ~3us of continuous execution. A chain of throwaway matmuls (dependent
    # only on an early memset) starts the ramp clock at ~0.3us so the first
    # real V matmuls already run at full clock.
    ones1_sb = persist.tile([1, 512], BF16, tag="ones1", name="ones1")
    nc.vector.memset(ones1_sb[:], 1.0)
    warm = ps_s.tile([128, 512], F32, tag="s", name="ps_warm")
    for _ in range(6):
        nc.tensor.matmul(warm[:], ones1_sb[:, 0:128], ones1_sb[:],
                         start=True, stop=True)

    # ---- persistent SBUF tiles + input DMAs, emitted in consumer order ----
    # channel blocks live in a middle free dim so each tensor loads in one
    # (or a few) large DMAs instead of 4x4 small ones
    w2 = ptile([128, 4, 3 * C], BF16, "w2")
    xt2 = ptile([128, 4, N], BF16, "xt2")
    xtq2 = ptile([128, 4, NQ], BF16, "xtq2")
    wcat_r = wcat_d.rearrange("(i p) c -> p i c", p=128)
    xt_r = xt_d.rearrange("(i p) n -> p i n", p=128)

    # DMA order follows consumption order: V (position-descending) needs
    # w-v + xt blk3 first; K projections need wk + k tables; then Q inputs;
    # then bias patterns (jt-descending, slot1 first); proj weights last.
    for i in range(4):  # first consumers: small DMAs for fast first arrival
        nc.sync.dma_start(w2[:, i, 2 * C:3 * C], wcat_r[:, i, 2 * C:3 * C])
        nc.sync.dma_start(xt2[:, i, 1536:2048], xt_r[:, i, 1536:2048])
    for blk in (2, 1, 0):  # V consumes position blocks descending
        nc.sync.dma_start(xt2[:, :, blk * 512:(blk + 1) * 512],
                          xt_r[:, :, blk * 512:(blk + 1) * 512])
    nc.sync.dma_start(w2[:, :, C:2 * C], wcat_r[:, :, C:2 * C])
    cos2k = ptile([128, N], BF16, "cos2k")
    nc.sync.dma_start(cos2k[:], cos2k_d[:])
    sin2k = ptile([128, N], BF16, "sin2k")
    nc.sync.dma_start(sin2k[:], sin2k_d[:])
    rotperm_sb = ptile([128, 128], BF16, "rotperm")
    nc.sync.dma_start(rotperm_sb[:], rotperm_d[:])
    cos2q = ptile([128, NQ], BF16, "cos2q")
    nc.sync.dma_start(cos2q[:], cos2q_d[:])
    sin2q = ptile([128, NQ], BF16, "sin2q")
    nc.sync.dma_start(sin2q[:], sin2q_d[:])
    nc.sync.dma_start(w2[:, :, 0:C], wcat_r[:, :, 0:C])
    nc.sync.dma_start(xtq2[:, :, :],
                      xtq_d.rearrange("(i p) n -> p i n", p=128))
    ident_sb = ptile([128, 128], BF16, "ident128")
    nc.sync.dma_start(ident_sb[:], ident_d[:])

    c8eye2 = ptile([128, H, 128], BF16, "c8eye2")
    nc.sync.dma_start(c8eye2[:], c8eye_d.rearrange("h p e -> p h e"))
    pats1_t = ptile([128, 16, 512], BF16, "pats1t")
    pats1_r = pats1_d.rearrange("j p q -> p j q")
    for j0 in (12, 8, 4, 0):
        nc.sync.dma_start(pats1_t[:, j0:j0 + 4, :], pats1_r[:, j0:j0 + 4, :])
    pats0_t = ptile([128, 8, 512], BF16, "pats0t")
    pats0_r = pats0_d.rearrange("j p q -> p j q")
    for j0 in (4, 0):
        nc.sync.dma_start(pats0_t[:, j0:j0 + 4, :], pats0_r[:, j0:j0 + 4, :])
    projw4 = ptile([128, 4, 512], BF16, "projw4")
    nc.sync.dma_start(projw4[:], projwt_d.rearrange("(t p) c -> p t c", p=128))
    biasrow_sb = ptile([1, 512], BF16, "biasrow")
    nc.sync.dma_start(biasrow_sb[:], biasrow_d[:])

    q2_sb = [ptile([128, NQ], BF16, f"q2_{t}") for t in range(4)]
    k2_sb = [ptile([128, N], BF16, f"k2_{t}") for t in range(4)]
    v_sb = [ptile([128, H * 65], BF16, f"v_{nt}") for nt in range(JT)]
    # transposed attention tiles [dims(2 heads x 64), 128 queries] per (t, qg)
    outT = [[ptile([128, 128], BF16, f"oT{t}_{qg}") for qg in range(8)]
            for t in range(4)]

    # ---- V projection, position-descending (av streams consume jt=15 first)
    def emit_v(nt):
        # alternate V psums between the (idle) score pool and ps_a: more
        # slots in flight -> the psum ring is no longer copy-latency bound
        pool = ps_s if nt % 2 == 0 else ps_a
        tag = "s" if nt % 2 == 0 else "a"
        psv = pool.tile([128, 512], F32, tag=tag, name="ps_v")
        for ci in range(4):
            nc.tensor.matmul(
                psv[:], xt2[:, ci, nt * 128:(nt + 1) * 128],
                w2[:, ci, 2 * C:3 * C],
                start=(ci == 0), stop=(ci == 3))
        vdst = v_sb[nt].rearrange("p (h e) -> p h e", e=65)
        # DVE is the prephase pacer and ACT is idle there: alternate the
        # psum->sbuf copies between them
        if nt % 2 == 0:
            nc.vector.tensor_copy(vdst[:, :, 0:64],
                                   psv.rearrange("p (h e) -> p h e", e=64))
        else:
            nc.scalar.activation(vdst[:, :, 0:64],
                                 psv.rearrange("p (h e) -> p h e", e=64),
                                 mybir.ActivationFunctionType.Copy)
        nc.gpsimd.memset(vdst[:, :, 64:65], 1.0)

    # ---- Q/K projections + RoPE (psums from ps_a: no contention with the
    # score psums when chunks are injected into the score phase) ----
    # QK chunk, software-pipelined in two parts so the rotate matmul never
    # waits in-order on its own chunk's psum->sbuf copy.
    qk_pend = []

    def qk_part2(st):
        qsb, cos_sb, sin_sb, dst_sb, c0 = st
        ps_r = ps_a.tile([128, 512], F32, tag="a", name="ps_r")
        nc.tensor.matmul(ps_r[:], rotperm_sb[:], qsb[:], start=True, stop=True)
        tc_c = tmp_pool.tile([128, 512], BF16, tag="rt", name="rt_c")
        nc.vector.tensor_mul(tc_c[:], qsb[:], cos_sb[:, c0:c0 + 512])
        tc_u = tmp_pool.tile([128, 512], F32, tag="ru", name="rt_u")
        nc.vector.tensor_mul(tc_u[:], ps_r[:], sin_sb[:, c0:c0 + 512])
        nc.gpsimd.tensor_add(dst_sb[:, c0:c0 + 512], tc_c[:], tc_u[:])

    def qk_flush():
        while qk_pend:
            qk_part2(qk_pend.pop(0))

    def qk_chunk(dt_tile, w_off, rhs_sb, cos_sb, sin_sb, dst_sb, c0,
                 act_copy=False, defer=False):
        # one projection matmul set; rotate_half applied afterwards as a
        # +-1 permutation matmul (replaces the rotated-weight projection)
        ps_q = ps_a.tile([128, 512], F32, tag="a", name="ps_q")
        for ci in range(4):
            nc.tensor.matmul(
                ps_q[:],
                w2[:, ci, w_off + dt_tile * 128: w_off + (dt_tile + 1) * 128],
                rhs_sb[:, ci, c0:c0 + 512],
                start=(ci == 0), stop=(ci == 3))
        qsb = tmp_pool.tile([128, 512], BF16, tag="qs", name="qsb")
        if act_copy:  # ACT is idle in the upfront QK phase; DVE is the pacer
            nc.scalar.activation(qsb[:], ps_q[:],
                                 mybir.ActivationFunctionType.Copy)
        else:
            nc.vector.tensor_copy(qsb[:], ps_q[:])
        if defer:  # prephase: rot matmul runs under the next chunk's cover
            qk_pend.append((qsb, cos_sb, sin_sb, dst_sb, c0))
            if len(qk_pend) > 1:
                qk_part2(qk_pend.pop(0))
        else:
            qk_part2((qsb, cos_sb, sin_sb, dst_sb, c0))

    def qk_fillers(t, act_copy=False, defer=False):
        """QK work for head pair t as thunks: k chunks (desc), then q."""
        thunks = []
        for ch in range(N // 512 - 1, -1, -1):
            thunks.append(lambda ch=ch: qk_chunk(
                t, C, xt2, cos2k, sin2k, k2_sb[t], ch * 512, act_copy, defer))
        for ch in range(NQ // 512):
            thunks.append(lambda ch=ch: qk_chunk(
                t, 0, xtq2, cos2q, sin2q, q2_sb[t], ch * 512, act_copy, defer))
        return thunks

    # Interleave QK(t0) chunks with the V tail so the PE rides out the
    # later xt-block DMAs instead of stalling on them.
    qk0 = qk_fillers(0, act_copy=True, defer=True)  # k desc, then q
    for nt in range(15, 7, -1):
        emit_v(nt)
    qk0.pop(0)()
    qk0.pop(0)()
    for nt in range(7, 3, -1):
        emit_v(nt)
    qk0.pop(0)()
    for nt in range(3, -1, -1):
        emit_v(nt)
    for th in qk0:
        th()
    qk_flush()

    # ---- scores + ALiBi bias + exp + attn@v (stationary-exp orientation) ----
    # The two 512-query slots of a head pair run as interleaved streams so
    # the PE always has the other stream's score matmuls to chew on while
    # ACT computes this stream's exp (in-order PE would otherwise stall on
    # every av group).
    class Stream:
        def __init__(self, t, slot):
            self.t, self.slot = t, slot
            self.jts = [jt for jt in range(JT - 1, -1, -1)
                        if QHI[t][slot][jt] > 0]
            self.pats = pats1_t if slot == 1 else pats0_t
            self.avp = [ps_a.tile([128, 4 * 65], F32, tag="a",
                                  name=f"av{t}_{slot}_{p}") for p in range(2)]
            self.ks = 0          # score tiles emitted
            self.ka = 0          # av groups emitted
            self.pending = []    # (jt, qhi, et) awaiting av emission
            self.normed = False

        def scores_left(self):
            return self.ks < len(self.jts)

        def drain_one(self):
            if self.pending:
                self.emit_av()
            if self.ka == len(self.jts) and not self.normed:
                self.normed = True
                self.emit_normalize()

        def drain_all(self):
            while self.pending:
                self.drain_one()
            self.drain_one()

        def emit_scores(self):
            t, slot = self.t, self.slot
            jt = self.jts[self.ks]
            qhi = QHI[t][slot][jt]
            qlo = min(QLO[slot][jt], qhi)
            ps = ps_s.tile([128, 1024], F32, tag="s", name="ps_sc")
            for p in range(2):
                if QHIH[t][slot][jt][p] == 0:
                    continue  # head fully out of radius: its av is skipped,
                              # so neither scores nor exp are needed
                h = 2 * t + p
                ks = k2_sb[t][64 * p:64 * (p + 1), jt * 128:(jt + 1) * 128]
                qbase = slot * 512
                # bias only matters up to this head's own radius: av skips
                # the columns beyond it, so their raw (unbiased) scores are
                # computed (the paired exp reads them) but never consumed
                bhi = min(qhi, QHIH[t][slot][jt][p])
                # One psum "zero region" (bank) per head-half: the first
                # matmul starts (lazy-zeroes) it, the last one stops it.
                if qlo >= bhi:  # no biased columns this head consumes
                    nc.tensor.matmul(
                        ps[:, p * 512:p * 512 + qhi], ks,
                        q2_sb[t][64 * p:64 * (p + 1), qbase:qbase + qhi],
                        start=True, stop=True, tile_position=(64 * p, 0))
                else:
                    if qlo > 0:
                        nc.tensor.matmul(
                            ps[:, p * 512:p * 512 + qlo], ks,
                            q2_sb[t][64 * p:64 * (p + 1), qbase:qbase + qlo],
                            start=True, stop=False, tile_position=(64 * p, 0))
                    nc.tensor.matmul(
                        ps[:, p * 512 + qlo:p * 512 + qhi], ks,
                        q2_sb[t][64 * p:64 * (p + 1), qbase + qlo:qbase + qhi],
                        start=(qlo == 0), stop=False, tile_position=(64 * p, 0))
                    nc.tensor.matmul(
                        ps[:, p * 512 + qlo:p * 512 + bhi], c8eye2[:, h, :],
                        self.pats[:, jt, qlo:bhi],
                        start=False, stop=True, tile_position=(0, 0))
            et = exp_pool.tile([128, 1024], BF16, tag="e", name="et")
            if QHIH[t][slot][jt][0] == 0:  # even half unwritten: odd only
                nc.scalar.activation(et[:, 512:512 + qhi],
                                     ps[:, 512:512 + qhi], Exp)
            elif qhi == 512:
                nc.scalar.activation(et[:], ps[:], Exp)
            else:
                psv_ap = ps.rearrange("p (h q) -> p h q", h=2)[:, :, 0:qhi]
                etv_ap = et.rearrange("p (h q) -> p h q", h=2)[:, :, 0:qhi]
                nc.scalar.activation(etv_ap, psv_ap, Exp)
            self.pending.append((jt, qhi, et))
            self.ks += 1

        def emit_av(self):
            t, slot = self.t, self.slot
            jt, qhi, et = self.pending.pop(0)
            first = (self.ka == 0)
            for p in range(2):
                h = 2 * t + p
                qhi_p = min(qhi, QHIH[t][slot][jt][p])
                vs = v_sb[jt][:, h * 65:(h + 1) * 65]
                for qg in range(4):
                    qw = min(128, qhi_p - 128 * qg)
                    if qw <= 0:
                        break
                    nc.tensor.matmul(
                        self.avp[p][0:qw, qg * 65:(qg + 1) * 65],
                        et[:, p * 512 + qg * 128:p * 512 + qg * 128 + qw],
                        vs,
                        start=(first and qg == 0),
                        stop=(jt == JSTOPH[t][slot][p][qg]),
                        skip_group_check=True)
            self.ka += 1

        def emit_normalize(self):
            t, slot = self.t, self.slot
            recs = []
            for p in range(2):
                rec = rec_pool.tile([128, 4], F32, tag="r", name="rec")
                nc.vector.reciprocal(
                    rec[:],
                    self.avp[p].rearrange("p (g e) -> p g e", e=65)[:, :, 64])
                recs.append(rec)
            for qg in range(4):
                att = att_pool.tile([128, 128], BF16, tag="t", name="att")
                for p in range(2):
                    nc.vector.tensor_scalar_mul(
                        att[:, p * 64:(p + 1) * 64],
                        self.avp[p][:, qg * 65:qg * 65 + 64],
                        recs[p][:, qg:qg + 1])
                if t == 3 and slot == 1:
                    # tail: PE transpose (+DVE copy) beats the serialized
                    # HWDGE xbar path on the critical path to the projection
                    ps_t = ps_a.tile([128, 128], BF16, tag="a", name="ps_t")
                    nc.tensor.transpose(ps_t[:], att[:], ident_sb[:])
                    nc.vector.tensor_copy(outT[t][slot * 4 + qg][:], ps_t[:])
                else:
                    nc.sync.dma_start_transpose(outT[t][slot * 4 + qg][:],
                                                att[:])

    def emit_proj(qg):
        psp = ps_a.tile([128, 512], F32, tag="a", name="ps_proj")
        for tt in range(4):
            nc.tensor.matmul(psp[:], outT[tt][qg][:], projw4[:, tt, :],
                             start=(tt == 0), stop=False)
        nc.tensor.matmul(psp[:], ones1_sb[:, 0:128], biasrow_sb[:],
                         start=False, stop=True)
        fin = fin_pool.tile([128, 512], F32, tag="f", name="fin")
        nc.scalar.activation(fin[:], psp[:], mybir.ActivationFunctionType.Copy)
        nc.sync.dma_start(out_d[qg * 128:(qg + 1) * 128, :], fin[:])

    prev_stream = None
    for t in range(4):
        # Fillers injected into score-phase PE slack (they use ps_a slots,
        # not the score psums): QK chunks of the next head pair, and for the
        # last pair's slot1 the first half of the output projection (whose
        # outT inputs - slot0 of every pair - are complete by then).
        for slot in range(2):
            if t < 3:
                fillers = qk_fillers(t + 1) if slot == 0 else fillers
            else:
                fillers = [] if slot == 0 else [
                    (lambda qg=qg: emit_proj(qg)) for qg in range(4)]
            n_tiles = len([jt for jt in range(JT) if QHI[t][slot][jt] > 0])
            inject_every = max(3, n_tiles // max(1, len(fillers) or 1))
            tiles_done = 0
            s = Stream(t, slot)
            # prologue: two tiles in flight, then drain the previous
            # stream's tail under their cover (hides its last exp latency)
            s.emit_scores()
            if s.scores_left():
                s.emit_scores()
            if prev_stream is not None:
                prev_stream.drain_all()
            tiles_done = s.ks
            while s.scores_left():
                s.emit_scores()
                while len(s.pending) > 5:  # five-deep exp/av stagger
                    s.drain_one()
                tiles_done += 1
                if (fillers and tiles_done % inject_every == 0
                        and tiles_done <= n_tiles - 2):
                    fillers.pop(0)()
            while len(s.pending) > 2:
                s.drain_one()
            prev_stream = s
            if t == 3 or slot == 1:
                for th in fillers:
                    th()
                fillers = []
                qk_flush()
    prev_stream.drain_all()

    # ---- second half of the output projection (slot1 positions) ----
    # pairs share one staging tile and one output DMA: fewer serialized
    # HWDGE windows on the final critical path
    for qg in (4,):
        fin2 = fin_pool.tile([128, 2, 512], F32, tag="f2", name="fin2")
        for sub in range(2):
            psp = ps_s.tile([128, 512], F32, tag="s", name="ps_proj")
            for tt in range(4):
                nc.tensor.matmul(psp[:], outT[tt][qg + sub][:],
                                 projw4[:, tt, :], start=(tt == 0), stop=False)
            nc.tensor.matmul(psp[:], ones1_sb[:, 0:128], biasrow_sb[:],
                             start=False, stop=True)
            nc.scalar.activation(fin2[:, sub], psp[:],
                                 mybir.ActivationFunctionType.Copy)
        nc.sync.dma_start(
            out_d[qg * 128:(qg + 2) * 128, :].rearrange(
                "(s p) c -> p s c", p=128), fin2[:])
    # last two outputs unpaired: their single-width DMAs overlap the copies
    # instead of waiting for both, shortening the final critical chain
    emit_proj(6)
    emit_proj(7)

    ctx.close()


@functools.lru_cache(maxsize=1)
def _graph():
    return _build_graph()


def kernel(x, qkv_w, proj_w, proj_b):
    global LAST_RESULT
    x = np.asarray(x, np.float32)
    qkv_w = np.asarray(qkv_w, np.float32)
    proj_w = np.asarray(proj_w, np.float32)
    proj_b = np.asarray(proj_b, np.float32)

    nc = _graph()
    shared, sin, cos = _shared_inputs(qkv_w, proj_w, proj_b)
    in_maps = [_core_inputs(c, x, shared, sin, cos) for c in range(NCORES)]
    trace = bool(int(os.environ.get("KERNEL_TRACE", "0")))
    res = bass_utils.run_bass_kernel_spmd(nc, in_maps,
                                          core_ids=list(range(NCORES)),
                                          trace=trace)
    LAST_RESULT = res
    out = np.zeros((B, N, C), np.float32)
    for c in range(NCORES):
        b, s = c // 2, c % 2
        blocks = _owned_blocks(s)
        o = np.asarray(res.results[c]["out"], np.float32)
        out[b, blocks[0] * 512:(blocks[0] + 1) * 512] = o[0:512]
        out[b, blocks[1] * 512:(blocks[1] + 1) * 512] = o[512:1024]
    return out


# revision 97
# speedup vs baseline: 1.0162x; 1.0002x over previous
"""Fused multi-head attention layer (RoPE + ALiBi + softmax + out-proj) on 8 TRN2 cores.

Sharding: core c -> (batch b = c//2, query-half s = c%2). Each core owns 1024
queries of its batch (two 512-blocks, interleaved for ALiBi load balance),
computes K/V for all 2048 positions, and writes a disjoint slice of the output.
No collectives. All 8 cores run one SPMD graph; per-core differences (which
query blocks, ALiBi band offsets) are encoded purely in host-prepared data.

v2 optimizations (vs baseline, TimelineSim 222.6us -> 153.7us):
- Score/bias/exp/AV column-trimmed to the ALiBi-needed query prefix per
  (head-pair, key-tile, slot); bias matmuls further restricted to the
  biased suffix via split accumulation groups; av and bias matmuls
  additionally trimmed to each head's own radius (the columns beyond it
  hold exp(<-30)~0 / are never consumed by the trimmed av).
- Both heads of a pair share one [128,1024] score PSUM -> one exp
  activation per (pair, jt, slot), halving ACT fixed overhead.
- AV uses stationary=exp-tile / moving=v (65 cols) instead of streaming
  512 query columns: ~2x less PE stream time; output lands [query, dim].
- Softmax normalization becomes a per-partition tensor_scalar multiply.
- Attention tiles transposed on the DMA engines (xbar), projection runs
  with 128-deep contraction (head pairs packed), proj bias folded in as a
  rank-1 ones-row matmul.
- RoPE rotate_half applied as a +-1 permutation matmul on the projected
  q/k instead of a second rotated-weight projection (drops 96 projection
  matmuls); the psum->sbuf hop also makes the cos-multiply all-bf16.
- Deep software pipelining: score streams run one (t, slot) at a time
  with a five-deep exp/av stagger, chained across stream boundaries; QK
  chunks of the next head pair and the first half of the projection are
  injected as fillers into score-phase PE slack (on the ps_a pool, so
  they never contend with score psums).
- Inputs land in a few large multi-dim DMAs ordered by first consumer;
  the tail transposes bypass the DMA queue via PE transpose; V-phase
  psums alternate between both psum pools so the ring is never
  copy-latency bound; the final output DMAs are paired to halve the
  serialized HWDGE windows on the tail; a throwaway warmup matmul chain
  starts the PE p-state ramp clock early so real matmuls never pay the
  cold-clock penalty; prephase psum->sbuf copies split across DVE/ACT
  and the prephase rotate matmuls deferred one chunk.

(fp8 DoubleRow for the bias matmul halves its PE cost in the cost model
and passes CoreSim numerically, but the axon PJRT execution path fails
on it at runtime, so it stays bf16.)
"""

import functools
import os
import sys

import numpy as np

sys.path.insert(0, "/opt/trn_rl_repo")

import ml_dtypes  # noqa: E402

import concourse.bass as bass  # noqa: E402
import concourse.tile as tile  # noqa: E402
from concourse import bacc, mybir, bass_utils  # noqa: E402

BF16 = mybir.dt.bfloat16
F32 = mybir.dt.float32
NPBF = ml_dtypes.bfloat16

B, N, C, H, D = 4, 2048, 512, 8, 64
NCORES = 8
NQ = 1024            # local queries per core
JT = N // 128        # 16 j-tiles of 128 key positions
T_CUT = 30.0         # ALiBi cutoff in logits: exp(-30) is negligible
SCALE = D ** -0.5

# c8_h = alibi_slope_h * MAX_BIAS = 2^-(h+1) * 8 = 2^(2-h)
C8 = [2.0 ** (2 - h) for h in range(H)]
# band reach (in key positions) per head
RADIUS = [T_CUT / c for c in C8]

# SPMD union bounds over the two cores sharing a slot index:
# slot0 owns blocks {0,1}*512, slot1 owns {2,3}*512.
I0MIN = [0, 1024]
I0MAX = [512, 1536]


def _qhi(t, slot, jt):
    """Needed query-column prefix of the [128 keys x 512 q] tile (pair union)."""
    return max(
        max(0, min(512, 128 * jt + 127 + int(RADIUS[h]) + 1 - I0MIN[slot]))
        for h in (2 * t, 2 * t + 1))


def _qlo(slot, jt):
    """First query column where ALiBi bias can be nonzero (union over cores)."""
    return max(0, min(512, 128 * jt + 1 - I0MAX[slot]))


QHI = [[[_qhi(t, s, jt) for jt in range(JT)] for s in range(2)] for t in range(4)]
QLO = [[_qlo(s, jt) for jt in range(JT)] for s in range(2)]
# per (t, slot, qg): last (smallest) jt in descending order that writes qg
JSTOP = [[[min(jt for jt in range(JT) if QHI[t][s][jt] > 128 * qg)
           for qg in range(4)] for s in range(2)] for t in range(4)]

# per-head widths: av matmuls beyond a head's own radius act on exp(<-30)=~0
# columns and are dropped (contribution ~1e-13; scores/bias/exp keep the
# pair width so the psum/activation structure is untouched)
QHIH = [[[[max(0, min(512, 128 * jt + 127 + int(RADIUS[2 * t + p]) + 1
                      - I0MIN[s])) for p in range(2)] for jt in range(JT)]
         for s in range(2)] for t in range(4)]
JSTOPH = [[[[min(jt for jt in range(JT) if QHIH[t][s][jt][p] > 128 * qg)
             for qg in range(4)] for p in range(2)] for s in range(2)]
          for t in range(4)]

LAST_RESULT = None  # test harness reads exec_time_ns from here


def _owned_blocks(s):
    # 512-query blocks of the batch owned by query-half s (balanced for ALiBi)
    return (0, 3) if s == 0 else (1, 2)


def _rope_tables():
    inv = 1.0 / (10000.0 ** (np.arange(0, D, 2, dtype=np.float32) / D))
    f = np.arange(N, dtype=np.float32)[:, None] * inv[None, :]
    sin = np.concatenate([np.sin(f), np.sin(f)], axis=-1).astype(np.float32)
    cos = np.concatenate([np.cos(f), np.cos(f)], axis=-1).astype(np.float32)
    return sin, cos  # [N, D]


def _shared_inputs(qkv_w, proj_w, proj_b):
    wqT = np.ascontiguousarray(qkv_w[0:C].T) * SCALE       # [C, C]
    wkT = np.ascontiguousarray(qkv_w[C:2 * C].T)
    wvT = np.ascontiguousarray(qkv_w[2 * C:3 * C].T)
    wcat = np.concatenate([wqT, wkT, wvT], axis=1).astype(NPBF)

    ident128 = np.eye(128, dtype=np.float32)

    # rotate_half as a +-1 permutation: out[i] = -in[32+i], out[32+i] = in[i]
    # per 64-dim head; lhsT layout [d_in, d_out].
    rotperm = np.zeros((128, 128), np.float32)
    for hh in range(2):
        for i in range(32):
            rotperm[hh * 64 + 32 + i, hh * 64 + i] = -1.0
            rotperm[hh * 64 + i, hh * 64 + 32 + i] = 1.0

    c8eye = np.zeros((H, 128, 128), np.float32)
    for h in range(H):
        np.fill_diagonal(c8eye[h], C8[h])

    sin, cos = _rope_tables()
    cos2k = np.tile(cos.T, (2, 1))                         # [128, N]
    sin2k = np.tile(sin.T, (2, 1))
    return {
        "wcat": wcat,
        "rotperm": rotperm.astype(NPBF),
        "ident128": ident128.astype(NPBF),
        "c8eye": c8eye.astype(NPBF),
        "projwt": np.ascontiguousarray(proj_w.T).astype(NPBF),
        "biasrow": proj_b[None, :].astype(NPBF),
        "cos2k": cos2k.astype(NPBF), "sin2k": sin2k.astype(NPBF),
    }, sin, cos


def _pats_for(i0):
    jl = np.arange(128, dtype=np.float32)[:, None]
    il = np.arange(512, dtype=np.float32)[None, :]
    return [np.minimum((jt * 128 + jl) - (i0 + il), 0.0).astype(NPBF)
            for jt in range(16)]


def _core_inputs(c, x, shared, sin, cos):
    b, s = c // 2, c % 2
    blocks = _owned_blocks(s)
    gi = np.concatenate([np.arange(blk * 512, (blk + 1) * 512) for blk in blocks])

    xt = np.ascontiguousarray(x[b].T)                      # [C, N]
    xtq = np.ascontiguousarray(x[b][gi].T)                 # [C, NQ]

    cos2q = np.tile(cos[gi].T, (2, 1))                     # [128, NQ]
    sin2q = np.tile(sin[gi].T, (2, 1))

    pats0 = np.stack(_pats_for(blocks[0] * 512)[:8])
    pats1 = np.stack(_pats_for(blocks[1] * 512))

    return {
        "xt": xt.astype(NPBF),
        "xtq": xtq.astype(NPBF),
        "cos2q": cos2q.astype(NPBF), "sin2q": sin2q.astype(NPBF),
        "pats0": pats0,
        "pats1": pats1,
        **shared,
    }


def _build_graph():
    nc = bacc.Bacc("TRN2", target_bir_lowering=False, debug=False,
                   num_devices=NCORES)

    xt_d = nc.dram_tensor("xt", [C, N], BF16, kind="ExternalInput").ap()
    xtq_d = nc.dram_tensor("xtq", [C, NQ], BF16, kind="ExternalInput").ap()
    wcat_d = nc.dram_tensor("wcat", [C, 3 * C], BF16, kind="ExternalInput").ap()
    rotperm_d = nc.dram_tensor("rotperm", [128, 128], BF16, kind="ExternalInput").ap()
    ident_d = nc.dram_tensor("ident128", [128, 128], BF16, kind="ExternalInput").ap()
    cos2q_d = nc.dram_tensor("cos2q", [128, NQ], BF16, kind="ExternalInput").ap()
    sin2q_d = nc.dram_tensor("sin2q", [128, NQ], BF16, kind="ExternalInput").ap()
    cos2k_d = nc.dram_tensor("cos2k", [128, N], BF16, kind="ExternalInput").ap()
    sin2k_d = nc.dram_tensor("sin2k", [128, N], BF16, kind="ExternalInput").ap()
    pats0_d = nc.dram_tensor("pats0", [8, 128, 512], BF16, kind="ExternalInput").ap()
    pats1_d = nc.dram_tensor("pats1", [16, 128, 512], BF16, kind="ExternalInput").ap()
    c8eye_d = nc.dram_tensor("c8eye", [H, 128, 128], BF16, kind="ExternalInput").ap()
    projwt_d = nc.dram_tensor("projwt", [C, C], BF16, kind="ExternalInput").ap()
    biasrow_d = nc.dram_tensor("biasrow", [1, 512], BF16, kind="ExternalInput").ap()
    out_d = nc.dram_tensor("out", [NQ, C], F32, kind="ExternalOutput").ap()

    with tile.TileContext(nc) as tc:
        _body(nc, tc, xt_d, xtq_d, wcat_d, rotperm_d, ident_d, cos2q_d,
              sin2q_d, cos2k_d, sin2k_d, pats0_d, pats1_d, c8eye_d, projwt_d,
              biasrow_d, out_d)
    nc.compile()
    return nc


def _body(nc, tc, xt_d, xtq_d, wcat_d, rotperm_d, ident_d, cos2q_d,
          sin2q_d, cos2k_d, sin2k_d, pats0_d, pats1_d, c8eye_d, projwt_d,
          biasrow_d, out_d):
    from contextlib import ExitStack
    ctx = ExitStack()
    persist = ctx.enter_context(tc.tile_pool(name="persist", bufs=1))
    tmp_pool = ctx.enter_context(tc.tile_pool(name="ropetmp", bufs=6))
    exp_pool = ctx.enter_context(tc.tile_pool(name="exp", bufs=10))
    fin_pool = ctx.enter_context(tc.tile_pool(name="final", bufs=2))
    att_pool = ctx.enter_context(tc.tile_pool(name="att", bufs=8))
    rec_pool = ctx.enter_context(tc.tile_pool(name="rec", bufs=4))
    # PSUM: ps_s = 2 bufs x [128,1024] f32 (2 banks each); ps_a = 4 bufs x
    # [128,512] f32 (1 bank each) shared by QKV-phase psums and AV accums.
    ps_s = ctx.enter_context(tc.tile_pool(name="ps_s", bufs=2, space="PSUM"))
    ps_a = ctx.enter_context(tc.tile_pool(name="ps_a", bufs=4, space="PSUM"))

    def ptile(shape, dtype, tag):
        return persist.tile(shape, dtype, tag=tag, name=tag)

    Exp = mybir.ActivationFunctionType.Exp

    # PE p-state warmup: the cost model ramps 0.65 -> 1.2 -> 2.4 GHz over
    # ~3us of continuous execution. A chain of throwaway matmuls (dependent
    # only on an early memset) starts the ramp clock at ~0.3us so the first
    # real V matmuls already run at full clock.
    ones1_sb = persist.tile([1, 512], BF16, tag="ones1", name="ones1")
    nc.vector.memset(ones1_sb[:], 1.0)
    warm = ps_s.tile([128, 512], F32, tag="s", name="ps_warm")
    for _ in range(6):
        nc.tensor.matmul(warm[:], ones1_sb[:, 0:128], ones1_sb[:],
                         start=True, stop=True)

    # ---- persistent SBUF tiles + input DMAs, emitted in consumer order ----
    # channel blocks live in a middle free dim so each tensor loads in one
    # (or a few) large DMAs instead of 4x4 small ones
    w2 = ptile([128, 4, 3 * C], BF16, "w2")
    xt2 = ptile([128, 4, N], BF16, "xt2")
    xtq2 = ptile([128, 4, NQ], BF16, "xtq2")
    wcat_r = wcat_d.rearrange("(i p) c -> p i c", p=128)
    xt_r = xt_d.rearrange("(i p) n -> p i n", p=128)

    # DMA order follows consumption order: V (position-descending) needs
    # w-v + xt blk3 first; K projections need wk + k tables; then Q inputs;
    # then bias patterns (jt-descending, slot1 first); proj weights last.
    for i in range(4):  # first consumers: small DMAs for fast first arrival
        nc.sync.dma_start(w2[:, i, 2 * C:3 * C], wcat_r[:, i, 2 * C:3 * C])
        nc.sync.dma_start(xt2[:, i, 1536:2048], xt_r[:, i, 1536:2048])
    for blk in (2, 1, 0):  # V consumes position blocks descending
        nc.sync.dma_start(xt2[:, :, blk * 512:(blk + 1) * 512],
                          xt_r[:, :, blk * 512:(blk + 1) * 512])
    nc.sync.dma_start(w2[:, :, C:2 * C], wcat_r[:, :, C:2 * C])
    cos2k = ptile([128, N], BF16, "cos2k")
    nc.sync.dma_start(cos2k[:], cos2k_d[:])
    sin2k = ptile([128, N], BF16, "sin2k")
    nc.sync.dma_start(sin2k[:], sin2k_d[:])
    rotperm_sb = ptile([128, 128], BF16, "rotperm")
    nc.sync.dma_start(rotperm_sb[:], rotperm_d[:])
    cos2q = ptile([128, NQ], BF16, "cos2q")
    nc.sync.dma_start(cos2q[:], cos2q_d[:])
    sin2q = ptile([128, NQ], BF16, "sin2q")
    nc.sync.dma_start(sin2q[:], sin2q_d[:])
    nc.sync.dma_start(w2[:, :, 0:C], wcat_r[:, :, 0:C])
    nc.sync.dma_start(xtq2[:, :, :],
                      xtq_d.rearrange("(i p) n -> p i n", p=128))
    ident_sb = ptile([128, 128], BF16, "ident128")
    nc.sync.dma_start(ident_sb[:], ident_d[:])

    c8eye2 = ptile([128, H, 128], BF16, "c8eye2")
    nc.sync.dma_start(c8eye2[:], c8eye_d.rearrange("h p e -> p h e"))
    pats1_t = ptile([128, 16, 512], BF16, "pats1t")
    pats1_r = pats1_d.rearrange("j p q -> p j q")
    for j0 in (12, 8, 4, 0):
        nc.sync.dma_start(pats1_t[:, j0:j0 + 4, :], pats1_r[:, j0:j0 + 4, :])
    pats0_t = ptile([128, 8, 512], BF16, "pats0t")
    pats0_r = pats0_d.rearrange("j p q -> p j q")
    for j0 in (4, 0):
        nc.sync.dma_start(pats0_t[:, j0:j0 + 4, :], pats0_r[:, j0:j0 + 4, :])
    projw4 = ptile([128, 4, 512], BF16, "projw4")
    nc.sync.dma_start(projw4[:], projwt_d.rearrange("(t p) c -> p t c", p=128))
    biasrow_sb = ptile([1, 512], BF16, "biasrow")
    nc.sync.dma_start(biasrow_sb[:], biasrow_d[:])

    q2_sb = [ptile([128, NQ], BF16, f"q2_{t}") for t in range(4)]
    k2_sb = [ptile([128, N], BF16, f"k2_{t}") for t in range(4)]
    v_sb = [ptile([128, H * 65], BF16, f"v_{nt}") for nt in range(JT)]
    # transposed attention tiles [dims(2 heads x 64), 128 queries] per (t, qg)
    outT = [[ptile([128, 128], BF16, f"oT{t}_{qg}") for qg in range(8)]
            for t in range(4)]

    # ---- V projection, position-descending (av streams consume jt=15 first)
    def emit_v(nt):
        # alternate V psums between the (idle) score pool and ps_a: more
        # slots in flight -> the psum ring is no longer copy-latency bound
        pool = ps_s if nt % 2 == 0 else ps_a
        tag = "s" if nt % 2 == 0 else "a"
        psv = pool.tile([128, 512], F32, tag=tag, name="ps_v")
        for ci in range(4):
            nc.tensor.matmul(
                psv[:], xt2[:, ci, nt * 128:(nt + 1) * 128],
                w2[:, ci, 2 * C:3 * C],
                start=(ci == 0), stop=(ci == 3))
        vdst = v_sb[nt].rearrange("p (h e) -> p h e", e=65)
        # DVE is the prephase pacer and ACT is idle there: alternate the
        # psum->sbuf copies between them
        if nt % 2 == 0:
            nc.vector.tensor_copy(vdst[:, :, 0:64],
                                   psv.rearrange("p (h e) -> p h e", e=64))
        else:
            nc.scalar.activation(vdst[:, :, 0:64],
                                 psv.rearrange("p (h e) -> p h e", e=64),
                                 mybir.ActivationFunctionType.Copy)
        nc.gpsimd.memset(vdst[:, :, 64:65], 1.0)

    # ---- Q/K projections + RoPE (psums from ps_a: no contention with the
    # score psums when chunks are injected into the score phase) ----
    # QK chunk, software-pipelined in two parts so the rotate matmul never
    # waits in-order on its own chunk's psum->sbuf copy.
    qk_pend = []

    def qk_part2(st):
        qsb, cos_sb, sin_sb, dst_sb, c0 = st
        ps_r = ps_a.tile([128, 512], F32, tag="a", name="ps_r")
        nc.tensor.matmul(ps_r[:], rotperm_sb[:], qsb[:], start=True, stop=True)
        tc_c = tmp_pool.tile([128, 512], BF16, tag="rt", name="rt_c")
        nc.vector.tensor_mul(tc_c[:], qsb[:], cos_sb[:, c0:c0 + 512])
        tc_u = tmp_pool.tile([128, 512], F32, tag="ru", name="rt_u")
        nc.vector.tensor_mul(tc_u[:], ps_r[:], sin_sb[:, c0:c0 + 512])
        nc.gpsimd.tensor_add(dst_sb[:, c0:c0 + 512], tc_c[:], tc_u[:])

    def qk_flush():
        while qk_pend:
            qk_part2(qk_pend.pop(0))

    def qk_chunk(dt_tile, w_off, rhs_sb, cos_sb, sin_sb, dst_sb, c0,
                 act_copy=False, defer=False):
        # one projection matmul set; rotate_half applied afterwards as a
        # +-1 permutation matmul (replaces the rotated-weight projection)
        ps_q = ps_a.tile([128, 512], F32, tag="a", name="ps_q")
        for ci in range(4):
            nc.tensor.matmul(
                ps_q[:],
                w2[:, ci, w_off + dt_tile * 128: w_off + (dt_tile + 1) * 128],
                rhs_sb[:, ci, c0:c0 + 512],
                start=(ci == 0), stop=(ci == 3))
        qsb = tmp_pool.tile([128, 512], BF16, tag="qs", name="qsb")
        if act_copy:  # ACT is idle in the upfront QK phase; DVE is the pacer
            nc.scalar.activation(qsb[:], ps_q[:],
                                 mybir.ActivationFunctionType.Copy)
        else:
            nc.vector.tensor_copy(qsb[:], ps_q[:])
        if defer:  # prephase: rot matmul runs under the next chunk's cover
            qk_pend.append((qsb, cos_sb, sin_sb, dst_sb, c0))
            if len(qk_pend) > 1:
                qk_part2(qk_pend.pop(0))
        else:
            qk_part2((qsb, cos_sb, sin_sb, dst_sb, c0))

    def qk_fillers(t, act_copy=False, defer=False):
        """QK work for head pair t as thunks: k chunks (desc), then q."""
        thunks = []
        for ch in range(N // 512 - 1, -1, -1):
            thunks.append(lambda ch=ch: qk_chunk(
                t, C, xt2, cos2k, sin2k, k2_sb[t], ch * 512, act_copy, defer))
        for ch in range(NQ // 512):
            thunks.append(lambda ch=ch: qk_chunk(
                t, 0, xtq2, cos2q, sin2q, q2_sb[t], ch * 512, act_copy, defer))
        return thunks

    # Interleave QK(t0) chunks with the V tail so the PE rides out the
    # later xt-block DMAs instead of stalling on them.
    qk0 = qk_fillers(0, act_copy=True, defer=True)  # k desc, then q
    for nt in range(15, 7, -1):
        emit_v(nt)
    qk0.pop(0)()
    qk0.pop(0)()
    for nt in range(7, 3, -1):
        emit_v(nt)
    qk0.pop(0)()
    for nt in range(3, -1, -1):
        emit_v(nt)
    for th in qk0:
        th()
    qk_flush()

    # ---- scores + ALiBi bias + exp + attn@v (stationary-exp orientation) ----
    # The two 512-query slots of a head pair run as interleaved streams so
    # the PE always has the other stream's score matmuls to chew on while
    # ACT computes this stream's exp (in-order PE would otherwise stall on
    # every av group).
    class Stream:
        def __init__(self, t, slot):
            self.t, self.slot = t, slot
            self.jts = [jt for jt in range(JT - 1, -1, -1)
                        if QHI[t][slot][jt] > 0]
            self.pats = pats1_t if slot == 1 else pats0_t
            self.avp = [ps_a.tile([128, 4 * 65], F32, tag="a",
                                  name=f"av{t}_{slot}_{p}") for p in range(2)]
            self.ks = 0          # score tiles emitted
            self.ka = 0          # av groups emitted
            self.pending = []    # (jt, qhi, et) awaiting av emission
            self.normed = False

        def scores_left(self):
            return self.ks < len(self.jts)

        def drain_one(self):
            if self.pending:
                self.emit_av()
            if self.ka == len(self.jts) and not self.normed:
                self.normed = True
                self.emit_normalize()

        def drain_all(self):
            while self.pending:
                self.drain_one()
            self.drain_one()

        def emit_scores(self):
            t, slot = self.t, self.slot
            jt = self.jts[self.ks]
            qhi = QHI[t][slot][jt]
            qlo = min(QLO[slot][jt], qhi)
            ps = ps_s.tile([128, 1024], F32, tag="s", name="ps_sc")
            for p in range(2):
                if QHIH[t][slot][jt][p] == 0:
                    continue  # head fully out of radius: its av is skipped,
                              # so neither scores nor exp are needed
                h = 2 * t + p
                ks = k2_sb[t][64 * p:64 * (p + 1), jt * 128:(jt + 1) * 128]
                qbase = slot * 512
                # bias only matters up to this head's own radius: av skips
                # the columns beyond it, so their raw (unbiased) scores are
                # computed (the paired exp reads them) but never consumed
                bhi = min(qhi, QHIH[t][slot][jt][p])
                # One psum "zero region" (bank) per head-half: the first
                # matmul starts (lazy-zeroes) it, the last one stops it.
                if qlo >= bhi:  # no biased columns this head consumes
                    nc.tensor.matmul(
                        ps[:, p * 512:p * 512 + qhi], ks,
                        q2_sb[t][64 * p:64 * (p + 1), qbase:qbase + qhi],
                        start=True, stop=True, tile_position=(64 * p, 0))
                else:
                    if qlo > 0:
                        nc.tensor.matmul(
                            ps[:, p * 512:p * 512 + qlo], ks,
                            q2_sb[t][64 * p:64 * (p + 1), qbase:qbase + qlo],
                            start=True, stop=False, tile_position=(64 * p, 0))
                    nc.tensor.matmul(
                        ps[:, p * 512 + qlo:p * 512 + qhi], ks,
                        q2_sb[t][64 * p:64 * (p + 1), qbase + qlo:qbase + qhi],
                        start=(qlo == 0), stop=False, tile_position=(64 * p, 0))
                    nc.tensor.matmul(
                        ps[:, p * 512 + qlo:p * 512 + bhi], c8eye2[:, h, :],
                        self.pats[:, jt, qlo:bhi],
                        start=False, stop=True, tile_position=(0, 0))
            et = exp_pool.tile([128, 1024], BF16, tag="e", name="et")
            if QHIH[t][slot][jt][0] == 0:  # even half unwritten: odd only
                nc.scalar.activation(et[:, 512:512 + qhi],
                                     ps[:, 512:512 + qhi], Exp)
            elif qhi == 512:
                nc.scalar.activation(et[:], ps[:], Exp)
            else:
                psv_ap = ps.rearrange("p (h q) -> p h q", h=2)[:, :, 0:qhi]
                etv_ap = et.rearrange("p (h q) -> p h q", h=2)[:, :, 0:qhi]
                nc.scalar.activation(etv_ap, psv_ap, Exp)
            self.pending.append((jt, qhi, et))
            self.ks += 1

        def emit_av(self):
            t, slot = self.t, self.slot
            jt, qhi, et = self.pending.pop(0)
            first = (self.ka == 0)
            for p in range(2):
                h = 2 * t + p
                qhi_p = min(qhi, QHIH[t][slot][jt][p])
                vs = v_sb[jt][:, h * 65:(h + 1) * 65]
                for qg in range(4):
                    qw = min(128, qhi_p - 128 * qg)
                    if qw <= 0:
                        break
                    nc.tensor.matmul(
                        self.avp[p][0:qw, qg * 65:(qg + 1) * 65],
                        et[:, p * 512 + qg * 128:p * 512 + qg * 128 + qw],
                        vs,
                        start=(first and qg == 0),
                        stop=(jt == JSTOPH[t][slot][p][qg]),
                        skip_group_check=True)
            self.ka += 1

        def emit_normalize(self):
            t, slot = self.t, self.slot
            recs = []
            for p in range(2):
                rec = rec_pool.tile([128, 4], F32, tag="r", name="rec")
                nc.vector.reciprocal(
                    rec[:],
                    self.avp[p].rearrange("p (g e) -> p g e", e=65)[:, :, 64])
                recs.append(rec)
            for qg in range(4):
                att = att_pool.tile([128, 128], BF16, tag="t", name="att")
                for p in range(2):
                    nc.vector.tensor_scalar_mul(
                        att[:, p * 64:(p + 1) * 64],
                        self.avp[p][:, qg * 65:qg * 65 + 64],
                        recs[p][:, qg:qg + 1])
                if t == 3 and slot == 1:
                    # tail: PE transpose (+DVE copy) beats the serialized
                    # HWDGE xbar path on the critical path to the projection
                    ps_t = ps_a.tile([128, 128], BF16, tag="a", name="ps_t")
                    nc.tensor.transpose(ps_t[:], att[:], ident_sb[:])
                    nc.vector.tensor_copy(outT[t][slot * 4 + qg][:], ps_t[:])
                else:
                    nc.sync.dma_start_transpose(outT[t][slot * 4 + qg][:],
                                                att[:])

    def emit_proj(qg):
        psp = ps_a.tile([128, 512], F32, tag="a", name="ps_proj")
        for tt in range(4):
            nc.tensor.matmul(psp[:], outT[tt][qg][:], projw4[:, tt, :],
                             start=(tt == 0), stop=False)
        nc.tensor.matmul(psp[:], ones1_sb[:, 0:128], biasrow_sb[:],
                         start=False, stop=True)
        fin = fin_pool.tile([128, 512], F32, tag="f", name="fin")
        nc.scalar.activation(fin[:], psp[:], mybir.ActivationFunctionType.Copy)
        nc.sync.dma_start(out_d[qg * 128:(qg + 1) * 128, :], fin[:])

    prev_stream = None
    for t in range(4):
        # Fillers injected into score-phase PE slack (they use ps_a slots,
        # not the score psums): QK chunks of the next head pair, and for the
        # last pair's slot1 the first half of the output projection (whose
        # outT inputs - slot0 of every pair - are complete by then).
        for slot in range(2):
            if t < 3:
                fillers = qk_fillers(t + 1) if slot == 0 else fillers
            else:
                fillers = [] if slot == 0 else [
                    (lambda qg=qg: emit_proj(qg)) for qg in range(4)]
            n_tiles = len([jt for jt in range(JT) if QHI[t][slot][jt] > 0])
            inject_every = max(3, n_tiles // max(1, len(fillers) or 1))
            tiles_done = 0
            s = Stream(t, slot)
            # prologue: two tiles in flight, then drain the previous
            # stream's tail under their cover (hides its last exp latency)
            s.emit_scores()
            if s.scores_left():
                s.emit_scores()
            if prev_stream is not None:
                prev_stream.drain_all()
            tiles_done = s.ks
            while s.scores_left():
                s.emit_scores()
                while len(s.pending) > 5:  # five-deep exp/av stagger
                    s.drain_one()
                tiles_done += 1
                if (fillers and tiles_done % inject_every == 0
                        and tiles_done <= n_tiles - 2):
                    fillers.pop(0)()
            while len(s.pending) > 2:
                s.drain_one()
            prev_stream = s
            if t == 3 or slot == 1:
                for th in fillers:
                    th()
                fillers = []
                qk_flush()
    prev_stream.drain_all()

    # ---- second half of the output projection (slot1 positions) ----
    # pairs share one staging tile and one output DMA: fewer serialized
    # HWDGE windows on the final critical path
    for qg in (4,):
        fin2 = fin_pool.tile([128, 2, 512], F32, tag="f2", name="fin2")
        for sub in range(2):
            psp = ps_s.tile([128, 512], F32, tag="s", name="ps_proj")
            for tt in range(4):
                nc.tensor.matmul(psp[:], outT[tt][qg + sub][:],
                                 projw4[:, tt, :], start=(tt == 0), stop=False)
            nc.tensor.matmul(psp[:], ones1_sb[:, 0:128], biasrow_sb[:],
                             start=False, stop=True)
            nc.scalar.activation(fin2[:, sub], psp[:],
                                 mybir.ActivationFunctionType.Copy)
        nc.sync.dma_start(
            out_d[qg * 128:(qg + 2) * 128, :].rearrange(
                "(s p) c -> p s c", p=128), fin2[:])
    # last two outputs unpaired: their single-width DMAs overlap the copies
    # instead of waiting for both, shortening the final critical chain
    emit_proj(6)
    emit_proj(7)

    ctx.close()


@functools.lru_cache(maxsize=1)
def _graph():
    return _build_graph()


def kernel(x, qkv_w, proj_w, proj_b):
    global LAST_RESULT
    x = np.asarray(x, np.float32)
    qkv_w = np.asarray(qkv_w, np.float32)
    proj_w = np.asarray(proj_w, np.float32)
    proj_b = np.asarray(proj_b, np.float32)

    nc = _graph()
    shared, sin, cos = _shared_inputs(qkv_w, proj_w, proj_b)
    in_maps = [_core_inputs(c, x, shared, sin, cos) for c in range(NCORES)]
    trace = bool(int(os.environ.get("KERNEL_TRACE", "0")))
    res = bass_utils.run_bass_kernel_spmd(nc, in_maps,
                                          core_ids=list(range(NCORES)),
                                          trace=trace)
    LAST_RESULT = res
    out = np.zeros((B, N, C), np.float32)
    for c in range(NCORES):
        b, s = c // 2, c % 2
        blocks = _owned_blocks(s)
        o = np.asarray(res.results[c]["out"], np.float32)
        out[b, blocks[0] * 512:(blocks[0] + 1) * 512] = o[0:512]
        out[b, blocks[1] * 512:(blocks[1] + 1) * 512] = o[512:1024]
    return out
